# revision 39
# baseline (speedup 1.0000x reference)
"""BiMambaBlock kernel for 8 TRN2 NeuronCores (Bass/Tile via PJRT).

Sharding: 8 cores = (modality i, direction dir, batch b) - each core runs the
full per-sequence pipeline on one channel-shuffled (and, for dir=1, L-flipped)
sequence x_i[b] of shape (96, 9216):
  LayerNorm -> RMSNorm -> fused causal-conv+input-projection -> SiLU
  -> B/C/dt projections -> selective scan (DVE tensor_tensor_scan over
  (d,n)-partition tiles, chunked along L with carried state) ->
  y = (scan + xc*Dp) * silu(z) -> 0.5 * output projection (+ residual on the
  fwd core). Host sums fwd/bwd partials and reshapes.

v3 engine plan (vs the first version):
  - packed scan layout: each scan tile holds 128 partitions = 4 states x
    32 channels (p = a*32+j covers n=4*g_n+a, d=32*g_d+j), so a chunk needs
    24 scan groups instead of 32; dt/dtx are staged to DRAM per chunk and
    replicated into the packed tiles by stride-0 DMA reads, B/C rows
    likewise; the y = sum_n h*C reduction uses three 0/1 reduction
    matrices on the PE with PSUM accumulation across all 24 groups.
  - this removes all per-(n) PE broadcast matmuls and Activation-engine
    PSUM->SBUF copies from the scan inner loop; DMA engines (otherwise
    idle) carry the broadcasts.
  - the B/C/dt projection path runs in bf16 (single PSUM tile + one bf16
    copy per sub-chunk), as do xc/g/dt/dtx/ya/yg; dA stays fp32; dA tiles
    are prefetched one scan-group ahead so Activation bursts don't stall
    the DVE backbone.
  - scan-state carry columns are DVE tensor_copies (SBUF->SBUF), not DMAs;
    the silu "+1" runs on Activation (Identity with bias).
  - bt=dtx*B and the scan run on DVE; h*C runs on Pool (the engine splits
    were tuned against the instruction-cost timeline simulator; Pool
    rejects TensorTensorScanArith and two-tensor TensorScalarPtr on real
    hardware, so its menu is plain tensor_tensor / tensor_scalar).
  - a post-compile pass collapses the alternating exp/ln activation-table
    loads into one load of the natural_log_exp_and_others set.

Self-contained: only needs numpy + jax + the concourse stack at
/opt/trn_rl_repo (present in the execution container).
"""
import sys
for _p in ("/opt/trn_rl_repo",):
    if _p not in sys.path:
        sys.path.insert(0, _p)
import numpy as np
from contextlib import ExitStack

import concourse.bass as bass
import concourse.bacc as bacc
import concourse.tile as tile
from concourse import mybir

F32 = mybir.dt.float32
F32R = mybir.dt.float32r
BF16 = mybir.dt.bfloat16
AF = mybir.ActivationFunctionType
OP = mybir.AluOpType

C, DI, N, R, K = 96, 192, 16, 6, 4
HH = WW = 96
L_FULL = HH * WW     # 9216
EPS = 1e-5

TC = 768             # time chunk
SUB = 384            # psum sub-chunk

# engine-assignment knobs for the scan inner loop: (n*2+hf) pairs with
# idx < HC_DVE_K run the h*C multiply on DVE, the rest on Pool; pairs with
# idx < BT_DVE_K run the dtx*B multiply on DVE, the rest on Pool.
HC_DVE_K = 0
BT_DVE_K = 32


def _fix_act_tables(nc):
    """Replace the alternating exp/ln table loads with a single load of a
    set containing every activation function the program uses."""
    from concourse.hw_specs import get_activation_tables
    used = set()
    for b in nc.main_func.blocks:
        for i in b.instructions:
            if isinstance(i, mybir.InstActivation):
                used.add(i.func)
    tables = get_activation_tables(nc.m.arch)
    target = None
    for idx, (name, funcs) in enumerate(tables.items()):
        if used <= funcs:
            target = idx
            break
    if target is None:
        return  # no single covering set; leave the compiler's placement
    for b in nc.main_func.blocks:
        keep_done = False
        new_insts = []
        for i in b.instructions:
            if isinstance(i, mybir.InstLoadActFuncSet):
                si = i.sync_info
                clean = si is None or (len(si.on_wait) == 0
                                       and len(si.on_update) == 0)
                if not keep_done or not clean:
                    i.act_func_set_id = target
                    keep_done = True
                    new_insts.append(i)
                # else: drop redundant load
            else:
                new_insts.append(i)
        b.instructions[:] = new_insts


def build_program(L=L_FULL, Tc=TC, pow_dA=False):
    NCHUNK = L // Tc
    NSUB = Tc // SUB
    TC3 = Tc + 3
    nc = bacc.Bacc("TRN2", target_bir_lowering=False, debug=False)

    x_in = nc.dram_tensor("x", [C, L], F32, kind="ExternalInput")
    wIN = nc.dram_tensor("wIN", [C, K * DI], F32, kind="ExternalInput")
    wZ = nc.dram_tensor("wZ", [C, DI], F32, kind="ExternalInput")
    wXPb = nc.dram_tensor("wXPb", [C, 2 * 38], BF16, kind="ExternalInput")
    wDTb = nc.dram_tensor("wDTb", [R, DI], BF16, kind="ExternalInput")
    wA2 = nc.dram_tensor("wA2", [128, 24], F32, kind="ExternalInput")
    wOUTb = nc.dram_tensor("wOUTb", [C, 2 * C], BF16, kind="ExternalInput")
    vec2 = nc.dram_tensor("vec2", [C, 8], F32, kind="ExternalInput")
    vec1 = nc.dram_tensor("vec1", [C, 3], F32, kind="ExternalInput")
    gate_in = nc.dram_tensor("gate", [1, 1], F32, kind="ExternalInput")
    red_in = nc.dram_tensor("red", [128, 3 * C], BF16, kind="ExternalInput")

    # ping-pong staging for the per-chunk B/C and dt/dtx rows
    # (replication/broadcast-DMA sources)
    bc_d = [nc.dram_tensor(f"bcrows{p}", [2 * N, Tc], BF16, kind="Internal")
            for p in range(2)]
    dt_d = [nc.dram_tensor(f"dtrows{p}", [2 * C, Tc], BF16, kind="Internal")
            for p in range(2)]
    dx_d = [nc.dram_tensor(f"dxrows{p}", [2 * C, Tc], BF16, kind="Internal")
            for p in range(2)]

    p_out = nc.dram_tensor("p", [C, L], F32, kind="ExternalOutput")

    with ExitStack() as ctx:
        tc = ctx.enter_context(tile.TileContext(nc))
        wp = ctx.enter_context(tc.tile_pool(name="wts", bufs=1))
        px = ctx.enter_context(tc.tile_pool(name="px", bufs=3))
        ph = ctx.enter_context(tc.tile_pool(name="ph", bufs=3))
        pt0 = ctx.enter_context(tc.tile_pool(name="pt0", bufs=2))
        pt1 = ctx.enter_context(tc.tile_pool(name="pt1", bufs=2))
        psp = ctx.enter_context(tc.tile_pool(name="psp", bufs=2))
        prow = ctx.enter_context(tc.tile_pool(name="prow", bufs=2))
        pxc = ctx.enter_context(tc.tile_pool(name="pxc", bufs=2))
        pg = ctx.enter_context(tc.tile_pool(name="pg", bufs=2))
        pdbl = ctx.enter_context(tc.tile_pool(name="pdbl", bufs=2))
        pdt = ctx.enter_context(tc.tile_pool(name="pdt", bufs=2))
        pdtx = ctx.enter_context(tc.tile_pool(name="pdtx", bufs=2))
        pbb = ctx.enter_context(tc.tile_pool(name="pbb", bufs=2))
        psc = ctx.enter_context(tc.tile_pool(name="psc", bufs=2))
        phh = ctx.enter_context(tc.tile_pool(name="phh", bufs=2))
        phc = ctx.enter_context(tc.tile_pool(name="phc", bufs=2))
        pst = ctx.enter_context(tc.tile_pool(name="pst", bufs=1))
        ptail = ctx.enter_context(tc.tile_pool(name="ptail", bufs=2))
        ppr = ctx.enter_context(tc.tile_pool(name="ppr", bufs=1))

        qh = ctx.enter_context(tc.tile_pool(name="qh", bufs=2, space="PSUM"))
        qdbl = ctx.enter_context(tc.tile_pool(name="qdbl", bufs=2,
                                              space="PSUM"))
        qy = ctx.enter_context(tc.tile_pool(name="qy", bufs=1, space="PSUM"))

        w_in = wp.tile([C, K * DI], F32); nc.sync.dma_start(w_in[:], wIN[:])
        w_z = wp.tile([C, DI], F32); nc.sync.dma_start(w_z[:], wZ[:])
        w_xpb = wp.tile([C, 2 * 38], BF16); nc.sync.dma_start(w_xpb[:],
                                                             wXPb[:])
        w_dtb = wp.tile([R, DI], BF16); nc.sync.dma_start(w_dtb[:], wDTb[:])
        w_a2 = wp.tile([128, 24], F32); nc.sync.dma_start(w_a2[:], wA2[:])
        w_outb = wp.tile([C, 2 * C], BF16); nc.sync.dma_start(w_outb[:],
                                                             wOUTb[:])
        v2 = wp.tile([C, 8], F32); nc.sync.dma_start(v2[:], vec2[:])
        v1 = wp.tile([C, 3], F32); nc.sync.dma_start(v1[:], vec1[:])
        gt = wp.tile([1, 1], F32); nc.sync.dma_start(gt[:], gate_in[:])

        # fp32r-rounded copies of the stationary matmul operands
        w_inr = wp.tile([C, K * DI], F32R); nc.scalar.copy(w_inr[:], w_in[:])
        w_zr = wp.tile([C, DI], F32R); nc.scalar.copy(w_zr[:], w_z[:])
        red_b = wp.tile([128, 3 * C], BF16)
        nc.sync.dma_start(red_b[:], red_in[:])

        ones_col = wp.tile([C, 1], F32); nc.vector.memset(ones_col[:], 1.0)
        ones_col_r = wp.tile([C, 1], F32R); nc.scalar.copy(ones_col_r[:],
                                                          ones_col[:])
        ones_row = wp.tile([1, C], F32); nc.vector.memset(ones_row[:], 1.0)
        ones_row_r = wp.tile([1, C], F32R); nc.scalar.copy(ones_row_r[:],
                                                          ones_row[:])
        epsc = wp.tile([1, 1], F32); nc.vector.memset(epsc[:], EPS)
        m2c = wp.tile([C, 1], F32); nc.vector.memset(m2c[:], -2.0)
        gate_c = wp.tile([C, 1], F32)
        qg = qh.tile([C, 1], F32, tag="a")
        nc.tensor.matmul(qg[:], ones_row[:], gt[:])
        nc.scalar.copy(gate_c[:], qg[:])

        # probes: absorb cross-engine waits (TensorScalarPtr ops: 1 wait slot)
        prv = ppr.tile([1, 8], F32)
        pra = ppr.tile([1, 8], F32)
        nc.vector.tensor_copy(prv[:, 0:1], v1[:1, 0:1])
        nc.vector.tensor_copy(prv[:, 1:2], v2[:1, 0:1])
        nc.vector.tensor_copy(prv[:, 2:3], gate_c[:1, 0:1])
        nc.scalar.copy(pra[:, 0:1], w_a2[:1, 0:1])
        nc.scalar.copy(pra[:, 1:2], v1[:1, 0:1])
        nc.scalar.copy(pra[:, 2:3], v2[:1, 0:1])

        st = pst.tile([128, 24], BF16)
        nc.vector.memset(st[:], 0.0)
        zero3 = wp.tile([C, 3], F32); nc.vector.memset(zero3[:], 0.0)

        def make_prologue(ci, chunks):
            """Emit-later closures for chunk ci's pre-scan pipeline. Each
            stage is emitted interleaved with the previous chunk's scan
            iterations so the in-order engine queues overlap them."""
            S = {"t0": ci * Tc}
            t0 = S["t0"]

            def s_dma():
                S["xt"] = px.tile([C, Tc], F32, tag="xt", name="xt")
                nc.sync.dma_start(S["xt"][:], x_in[:, t0:t0 + Tc])

            def s_sq():
                S["sq"] = pt0.tile([C, Tc], F32R, tag="sql", name="sq")
                nc.scalar.activation(S["sq"][:], S["xt"][:], AF.Square)

            def s_stats():
                S["m_"] = prow.tile([1, Tc], F32R, tag="m", bufs=2, name="m_")
                S["var_"] = prow.tile([1, Tc], F32, tag="var", bufs=1,
                                      name="var_")
                mm_ = prow.tile([1, Tc], F32, tag="mm", bufs=1, name="mm_")
                for si in range(NSUB):
                    o = si * SUB
                    s1 = qh.tile([1, SUB], F32, tag="a", name="s1")
                    nc.tensor.matmul(s1[:], ones_col[:], S["xt"][:, o:o + SUB])
                    nc.vector.tensor_scalar_mul(
                        S["m_"][:, o:o + SUB], s1[:], 1.0 / C)
                    s2 = qh.tile([1, SUB], F32, tag="a", name="s2")
                    nc.tensor.matmul(s2[:], ones_col_r[:],
                                     S["sq"][:, o:o + SUB])
                    nc.vector.tensor_tensor(
                        mm_[:, o:o + SUB], S["m_"][:, o:o + SUB].bitcast(F32),
                        S["m_"][:, o:o + SUB].bitcast(F32), op=OP.mult)
                    nc.vector.scalar_tensor_tensor(
                        S["var_"][:, o:o + SUB], s2[:], 1.0 / C,
                        mm_[:, o:o + SUB], op0=OP.mult, op1=OP.subtract)

            def s_rstd():
                lnv = prow.tile([1, Tc], F32, tag="lnv", bufs=1, name="lnv")
                S["rstd"] = prow.tile([1, Tc], F32R, tag="rstd", bufs=2,
                                      name="rstd")
                nc.scalar.activation(lnv[:], S["var_"][:], AF.Ln,
                                     bias=epsc[:, 0:1])
                nc.scalar.activation(S["rstd"][:], lnv[:], AF.Exp, scale=-0.5)

            def s_xn():
                S["xn"] = pt1.tile([C, Tc], F32, tag="xn", name="xn")
                for si in range(NSUB):
                    o = si * SUB
                    mb = qh.tile([C, SUB], F32, tag="a", name="mb")
                    nc.tensor.matmul(mb[:], ones_row_r[0:1, :],
                                     S["m_"][:, o:o + SUB])
                    nc.vector.tensor_tensor(S["xn"][:, o:o + SUB],
                                            S["xt"][:, o:o + SUB],
                                            mb[:], op=OP.subtract)
                    rb = qh.tile([C, SUB], F32, tag="a", name="rb")
                    nc.tensor.matmul(rb[:], ones_row_r[0:1, :],
                                     S["rstd"][:, o:o + SUB])
                    nc.vector.tensor_tensor(S["xn"][:, o:o + SUB],
                                            S["xn"][:, o:o + SUB],
                                            rb[:], op=OP.mult)

            def s_ln():
                S["ln_t"] = pt1.tile([C, Tc], F32, tag="ln", name="ln_t")
                nc.scalar.activation(S["ln_t"][:], S["xn"][:], AF.Identity,
                                     bias=v1[:, 1:2], scale=v1[:, 0:1])
                S["lsq"] = pt0.tile([C, Tc], F32R, tag="sql", name="lsq")
                nc.scalar.activation(S["lsq"][:], S["ln_t"][:], AF.Square)

            def s_rr():
                lnr = prow.tile([1, Tc], F32, tag="lnr", bufs=1, name="lnr")
                S["rr"] = prow.tile([1, Tc], F32R, tag="rr", bufs=2, name="rr")
                for si in range(NSUB):
                    o = si * SUB
                    s3 = qh.tile([1, SUB], F32, tag="a", name="s3")
                    nc.tensor.matmul(s3[:], ones_col_r[:],
                                     S["lsq"][:, o:o + SUB])
                    nc.scalar.activation(lnr[:, o:o + SUB], s3[:],
                                         AF.Ln, scale=1.0 / C,
                                         bias=epsc[:, 0:1])
                    nc.scalar.activation(S["rr"][:, o:o + SUB],
                                         lnr[:, o:o + SUB], AF.Exp, scale=-0.5)

            def s_h():
                h_t = ph.tile([C, TC3], F32R, tag="h", name="h_t")
                S["h_t"] = h_t
                if ci == 0:
                    nc.scalar.copy(h_t[:, 0:3], zero3[:])
                else:
                    hp = chunks[ci - 1]["h_t"]
                    nc.vector.tensor_copy(h_t[:, 0:3],
                                          hp[:, Tc:Tc + 3].bitcast(F32))
                nc.vector.tensor_copy(prv[:, 3:4], S["ln_t"][:1, 0:1])
                for si in range(NSUB):
                    o = si * SUB
                    rrb = qh.tile([C, SUB], F32, tag="a", name="rrb")
                    nc.tensor.matmul(rrb[:], ones_row_r[0:1, :],
                                     S["rr"][:, o:o + SUB])
                    nc.vector.scalar_tensor_tensor(
                        h_t[:, 3 + o:3 + o + SUB], S["ln_t"][:, o:o + SUB],
                        v1[:, 2:3], rrb[:], op0=OP.mult, op1=OP.mult)

            def make_conv(hf, si):
                def s_conv():
                    if "xc_h" not in S:
                        S["xc_h"] = [None, None]
                        S["g_h"] = [None, None]
                    if S["xc_h"][hf] is None:
                        S["xc_h"][hf] = pxc.tile([C, Tc], BF16, tag=f"xc{hf}",
                                                 name=f"xc{hf}")
                        S["g_h"][hf] = pg.tile([C, Tc], BF16, tag=f"g{hf}",
                                               name=f"g{hf}")
                    xc = S["xc_h"][hf]
                    g = S["g_h"][hf]
                    h_t = S["h_t"]
                    if True:
                        o = si * SUB
                        ps = qh.tile([C, SUB], F32, tag="a", name="psc1")
                        for k in range(K):
                            nc.tensor.matmul(
                                ps[:],
                                w_inr[:, k * DI + hf * C:k * DI + hf * C + C],
                                h_t[:, o + k:o + k + SUB],
                                start=(k == 0), stop=(k == K - 1))
                        # silu(p+cb) = (p+cb) / (1+exp(-(p+cb)))
                        e1 = psp.tile([C, SUB], F32, tag="sg1", name="e1")
                        nc.scalar.activation(e1[:], ps[:], AF.Exp, scale=-1.0,
                                             bias=v2[:, 6 + hf:7 + hf])
                        f1 = psp.tile([C, SUB], F32, tag="sg3", name="f1")
                        nc.scalar.activation(f1[:], e1[:], AF.Identity,
                                             bias=1.0)
                        r1 = psp.tile([C, SUB], F32, tag="sg2", name="r1")
                        nc.vector.reciprocal_approx_fast(r1[:], f1[:])
                        nc.vector.scalar_tensor_tensor(
                            xc[:, o:o + SUB], ps[:], v2[:, hf:hf + 1], r1[:],
                            op0=OP.add, op1=OP.mult)
                        ps2 = qh.tile([C, SUB], F32, tag="a", name="psc2")
                        nc.tensor.matmul(ps2[:], w_zr[:, hf * C:hf * C + C],
                                         h_t[:, o + 3:o + 3 + SUB])
                        e2 = psp.tile([C, SUB], F32, tag="sg1", name="e2")
                        nc.scalar.activation(e2[:], ps2[:], AF.Exp, scale=-1.0)
                        f2 = psp.tile([C, SUB], F32, tag="sg3", name="f2")
                        nc.scalar.activation(f2[:], e2[:], AF.Identity,
                                             bias=1.0)
                        r2 = psp.tile([C, SUB], F32, tag="sg2", name="r2")
                        nc.vector.reciprocal_approx_fast(r2[:], f2[:])
                        nc.vector.tensor_tensor(g[:, o:o + SUB], ps2[:], r2[:],
                                                op=OP.mult)
                return s_conv

            def make_dbl(si):
                # B/C/dt projections in one PSUM tile; bf16 copy. Matmul
                # outputs must start at partition 0 or 32 and match the
                # lhsT base, so: rows [0:32) = dt-proj + a redundant B/C
                # prefix (keeps every copied row defined), rows [32:64) =
                # B/C. dt-proj = dbc[0:R], B = dbc[32:48], C = dbc[48:64].
                def s_dbl():
                    if "dbc" not in S:
                        S["dbc"] = pdbl.tile([64, Tc], BF16, tag="dbc",
                                             name="dbc")
                    o = si * SUB
                    ps = qdbl.tile([64, SUB], F32, tag="d", name="psdbl")
                    for lo, src_lo in ((0, 0), (32, R)):
                        for hf in range(2):
                            nc.tensor.matmul(
                                ps[lo:lo + 32, :],
                                w_xpb[:, hf * 38 + src_lo:
                                      hf * 38 + src_lo + 32],
                                S["xc_h"][hf][:, o:o + SUB],
                                start=(hf == 0), stop=(hf == 1))
                    nc.scalar.copy(S["dbc"][:, o:o + SUB], ps[:])
                    if si == NSUB - 1:
                        # stage B/C rows to DRAM for the broadcast reads
                        nc.sync.dma_start(bc_d[ci % 2][:, :],
                                          S["dbc"][32:64, :])
                return s_dbl

            def make_dt(hf):
                def s_dt():
                    if "dt_h" not in S:
                        S["dt_h"] = [None, None]
                    S["dt_h"][hf] = pdt.tile([C, Tc], BF16, tag=f"dt{hf}",
                                             name=f"dt{hf}")
                    for si in range(NSUB):
                        o = si * SUB
                        ps = qh.tile([C, SUB], F32, tag="a", name="psdt")
                        nc.tensor.matmul(ps[:], w_dtb[:, hf * C:hf * C + C],
                                         S["dbc"][0:R, o:o + SUB])
                        # dt projections sit near dtb ~ -4, so u=e^v < 0.03
                        # and softplus(v) = u - u^2/2 + O(u^3); store
                        # dt2 = (u-2)*u = -2*softplus(v), the -2 is folded
                        # into wA2 and red host-side (the scan is linear).
                        # Two separate Exp tiles: the hardware TensorScalarPtr
                        # mis-reads when in0 and in1 are the same AP.
                        ex = psp.tile([C, SUB], F32, tag="spe", name="ex")
                        nc.scalar.activation(ex[:], ps[:], AF.Exp,
                                             bias=v2[:, 2 + hf:3 + hf])
                        exb = psp.tile([C, SUB], F32, tag="spe2", name="exb")
                        nc.scalar.activation(exb[:], ps[:], AF.Exp,
                                             bias=v2[:, 2 + hf:3 + hf])
                        nc.vector.scalar_tensor_tensor(
                            S["dt_h"][hf][:, o:o + SUB], ex[:], m2c[:, 0:1],
                            exb[:], op0=OP.add, op1=OP.mult)
                    nc.sync.dma_start(dt_d[ci % 2][hf * C:(hf + 1) * C, :],
                                      S["dt_h"][hf][:])
                return s_dt

            def make_dtx(hf):
                def s_dtx():
                    dx = pdtx.tile([C, Tc], BF16, tag=f"dtx{hf}",
                                   name=f"dtx{hf}")
                    nc.vector.tensor_tensor(dx[:], S["dt_h"][hf][:],
                                            S["xc_h"][hf][:], op=OP.mult)
                    nc.sync.dma_start(dx_d[ci % 2][hf * C:(hf + 1) * C, :],
                                      dx[:])
                return s_dtx

            def _rep_from(dram, r0):
                # (128,Tc) <- rows [r0, r0+32) of `dram`, each row at the 4
                # partitions a*32+j (a = n-subindex, j = d-subindex)
                sl = dram[r0:r0 + 32, :]
                return bass.AP(tensor=sl.tensor, offset=sl.offset,
                               ap=[[0, 4]] + [list(a) for a in sl.ap])

            def make_rep(hf):
                def s_rep():
                    if "dtp" not in S:
                        S["dtp"] = [[None] * 3, [None] * 3]
                        S["dxp"] = [[None] * 3, [None] * 3]
                    for g_d in range(3):
                        tp = pdt.tile([128, Tc], BF16, tag=f"dtp{hf}{g_d}",
                                      name=f"dtp{hf}{g_d}")
                        nc.sync.dma_start(
                            tp[:], _rep_from(dt_d[ci % 2], hf * C + 32 * g_d))
                        S["dtp"][hf][g_d] = tp
                        xp = pdtx.tile([128, Tc], BF16, tag=f"dxp{hf}{g_d}",
                                       name=f"dxp{hf}{g_d}")
                        nc.sync.dma_start(
                            xp[:], _rep_from(dx_d[ci % 2], hf * C + 32 * g_d))
                        S["dxp"][hf][g_d] = xp
                return s_rep

            early = [s_dma, s_sq, s_stats, s_rstd, s_xn, s_ln, s_rr, s_h]
            late = [make_conv(0, 0), make_conv(0, 1), make_conv(1, 0),
                    make_conv(1, 1), make_dbl(0), make_dbl(1),
                    make_dt(0), make_dt(1), make_dtx(0), make_dtx(1),
                    make_rep(0), make_rep(1)]
            return S, early, late

        def emit_scan_and_tail(S, ci, nxt_stages, nxt_S=None):
            t0 = S["t0"]
            xt, xc_h, g_h = S["xt"], S["xc_h"], S["g_h"]
            dtp, dxp = S["dtp"], S["dxp"]
            yps = [[qy.tile([C, SUB], F32, tag=f"y{hf}_{si}",
                            name=f"y{hf}_{si}")
                    for si in range(NSUB)] for hf in range(2)]

            def emit_bc(g_n):
                # (128,Tc) b/c tiles: rows 4*g_n+a at partitions a*32+j
                bb = pbb.tile([128, Tc], BF16, tag="bb", name="bb", bufs=4)
                cb = pbb.tile([128, Tc], BF16, tag="cb", name="cb", bufs=4)
                src_ = bc_d[ci % 2]
                for dst, r0 in ((bb, 4 * g_n), (cb, N + 4 * g_n)):
                    sl = src_[r0:r0 + 4, :]
                    ap = [list(sl.ap[0]), [0, 32], list(sl.ap[1])]
                    nc.sync.dma_start(dst[:], bass.AP(
                        tensor=sl.tensor, offset=sl.offset, ap=ap))
                return bb, cb

            iters = [(g_n, hf, g_d) for g_n in range(4) for hf in range(2)
                     for g_d in range(3)]

            def emit_dA(it):
                g_n, hf, g_d = iters[it]
                dA = psc.tile([128, Tc], F32, tag="dA", name="dA", bufs=4)
                nc.scalar.activation(dA[:], dtp[hf][g_d][:], AF.Exp,
                                     scale=w_a2[:, it:it + 1])
                return dA

            S["emit_bc"] = emit_bc
            S["emit_dA"] = emit_dA
            bcq = [S.pop("bc0", None) or emit_bc(0), emit_bc(1)]
            pre = S.pop("dA01", None)
            dAq = pre if pre else [emit_dA(0), emit_dA(1)]
            dAq.append(emit_dA(2))
            for it, (g_n, hf, g_d) in enumerate(iters):
                if g_d == 0 and hf == 0:
                    bb, cb = bcq.pop(0)
                    if g_n + 2 < 4:
                        bcq.append(emit_bc(g_n + 2))
                dA = dAq.pop(0)
                if it + 3 < 24:
                    dAq.append(emit_dA(it + 3))
                bt = psc.tile([128, Tc], BF16, tag="bt", name="bt", bufs=4)
                nc.vector.tensor_tensor(bt[:], dxp[hf][g_d][:], bb[:],
                                        op=OP.mult)
                ht = phh.tile([128, Tc], BF16, tag="ht", name="ht", bufs=4)
                nc.vector.tensor_tensor_scan(
                    ht[:], dA[:], bt[:], st[:, it:it + 1],
                    op0=OP.mult, op1=OP.add)
                nc.vector.tensor_copy(st[:, it:it + 1], ht[:, Tc - 1:Tc])
                hc = phc.tile([128, Tc], BF16, tag="hc", name="hc", bufs=4)
                nc.gpsimd.tensor_tensor(hc[:], ht[:], cb[:], op=OP.mult)
                first = (g_n == 0 and g_d == 0)
                last = (g_n == 3 and g_d == 2)
                for si in range(NSUB):
                    o = si * SUB
                    nc.tensor.matmul(yps[hf][si][:],
                                     red_b[:, g_d * C:(g_d + 1) * C],
                                     hc[:, o:o + SUB],
                                     start=first, stop=last,
                                     skip_group_check=True)
                if nxt_stages:
                    nxt_stages.pop(0)()

            while nxt_stages:
                nxt_stages.pop(0)()
            yg_h = []
            for hf in range(2):
                ya = ptail.tile([C, Tc], BF16, tag=f"ya{hf}", name=f"ya{hf}",
                                bufs=1)
                for si in range(NSUB):
                    o = si * SUB
                    nc.vector.scalar_tensor_tensor(
                        ya[:, o:o + SUB], xc_h[hf][:, o:o + SUB],
                        v2[:, 4 + hf:5 + hf], yps[hf][si][:],
                        op0=OP.mult, op1=OP.add)
                yg = ptail.tile([C, Tc], BF16, tag=f"yg{hf}", name=f"yg{hf}",
                                bufs=1)
                nc.gpsimd.tensor_tensor(yg[:], ya[:], g_h[hf][:], op=OP.mult)
                yg_h.append(yg)
            for si in range(NSUB):
                o = si * SUB
                pso = qh.tile([C, SUB], F32, tag="a", name="pso")
                for hf in range(2):
                    nc.tensor.matmul(pso[:], w_outb[:, hf * C:hf * C + C],
                                     yg_h[hf][:, o:o + SUB],
                                     start=(hf == 0), stop=(hf == 1))
                ot = ptail.tile([C, SUB], F32, tag="ot", name="ot")
                nc.vector.scalar_tensor_tensor(
                    ot[:], xt[:, o:o + SUB], gate_c[:, 0:1], pso[:],
                    op0=OP.mult, op1=OP.add)
                nc.sync.dma_start(p_out[:, t0 + o:t0 + o + SUB], ot[:])

        chunks = {}
        S0, early0, late0 = make_prologue(0, chunks)
        chunks[0] = S0
        for f in early0 + late0:
            f()
        pend_late = {}
        if NCHUNK > 1:
            S1, early1, late1 = make_prologue(1, chunks)
            chunks[1] = S1
            for f in early1:
                f()
            pend_late[1] = late1
        for ci in range(NCHUNK):
            stages = list(pend_late.pop(ci + 1, []))
            if ci + 2 < NCHUNK:
                S2, early2, late2 = make_prologue(ci + 2, chunks)
                chunks[ci + 2] = S2
                stages += early2
                pend_late[ci + 2] = late2
            emit_scan_and_tail(chunks[ci], ci, stages,
                               chunks.get(ci + 1))
            chunks.pop(ci - 1, None)

    nc.compile()
    _fix_act_tables(nc)
    return nc


# ---------------------------------------------------------------- host side

def shuffle_channels(x):
    c = x.shape[0]
    return x.reshape(2, c // 2, -1).transpose(1, 0, 2).reshape(c, -1)


def _bf16():
    try:
        import ml_dtypes
        return ml_dtypes.bfloat16
    except Exception:
        import jax.numpy as _jnp
        return _jnp.bfloat16


def pack_core_inputs(i, dr, b, x1, x2, inw, convw, convb, xpw, dtw, dtb,
                     Alog, Dp, outw, rmsw, lnw, lnb):
    xs = x1 if i == 0 else x2
    x = shuffle_channels(np.asarray(xs[b], np.float32))
    if dr == 1:
        x = x[:, ::-1]
    x = np.ascontiguousarray(x)

    inw_i = np.asarray(inw[i], np.float32)
    cw = np.asarray(convw[i, dr], np.float32)
    cb = np.asarray(convb[i, dr], np.float32)
    xp = np.asarray(xpw[i, dr], np.float32)
    dw = np.asarray(dtw[i, dr], np.float32)
    db = np.asarray(dtb[i, dr], np.float32)
    Av = -np.exp(np.asarray(Alog[i, dr], np.float32))
    Dv = np.asarray(Dp[i, dr], np.float32)
    ow = np.asarray(outw[i], np.float32)

    wIN = np.empty((C, K * DI), np.float32)
    inw_x = inw_i[:DI]
    for k in range(K):
        wIN[:, k * DI:(k + 1) * DI] = (cw[:, k][:, None] * inw_x).T
    wZ = np.ascontiguousarray(inw_i[DI:].T)
    bf16 = _bf16()
    wXP = np.empty((C, 2 * 38), np.float32)
    for hf in range(2):
        wXP[:, hf * 38:(hf + 1) * 38] = xp[:, hf * C:(hf + 1) * C].T
    wXPb = wXP.astype(bf16)
    wDTb = np.ascontiguousarray(dw.T).astype(bf16)
    # packed-layout A scales: column it=(g_n*6+hf*3+g_d) holds, at
    # partition p=a*32+j, the A value for n=4*g_n+a, d=32*g_d+j
    wA2 = np.empty((128, 24), np.float32)
    for g_n in range(4):
        for hf in range(2):
            for g_d in range(3):
                it = g_n * 6 + hf * 3 + g_d
                for a in range(4):
                    for j in range(32):
                        # -0.5 compensates the -2-scaled dt2 series
                        wA2[a * 32 + j, it] = -0.5 * Av[hf * C + 32 * g_d + j,
                                                        4 * g_n + a]
    wOUT = np.empty((C, 2 * C), np.float32)
    for hf in range(2):
        wOUT[:, hf * C:(hf + 1) * C] = 0.5 * ow[:, hf * C:(hf + 1) * C].T
    wOUTb = wOUT.astype(bf16)
    vec2 = np.ascontiguousarray(
        np.stack([cb[:C], cb[C:], db[:C], db[C:], Dv[:C], Dv[C:],
                  -cb[:C], -cb[C:]], axis=1), dtype=np.float32)
    vec1 = np.ascontiguousarray(
        np.stack([np.asarray(lnw[i], np.float32),
                  np.asarray(lnb[i], np.float32),
                  np.asarray(rmsw[i], np.float32)], axis=1), dtype=np.float32)
    gate = np.array([[1.0 if dr == 0 else 0.0]], np.float32)
    red = np.zeros((128, 3 * C), np.float32)
    for g_d in range(3):
        for a in range(4):
            for j in range(32):
                red[a * 32 + j, g_d * C + 32 * g_d + j] = -0.5
    red = red.astype(bf16)
    return {
        "x": x, "wIN": wIN, "wZ": wZ, "wXPb": wXPb, "wDTb": wDTb,
        "wA2": wA2, "wOUTb": wOUTb, "vec2": vec2, "vec1": vec1,
        "gate": gate, "red": red,
    }


def make_in_maps(inputs):
    args = dict(
        x1=np.asarray(inputs["x1"], np.float32),
        x2=np.asarray(inputs["x2"], np.float32),
        inw=np.asarray(inputs["inw"], np.float32),
        convw=np.asarray(inputs["convw"], np.float32),
        convb=np.asarray(inputs["convb"], np.float32),
        xpw=np.asarray(inputs["xpw"], np.float32),
        dtw=np.asarray(inputs["dtw"], np.float32),
        dtb=np.asarray(inputs["dtb"], np.float32),
        Alog=np.asarray(inputs["Alog"], np.float32),
        Dp=np.asarray(inputs["Dp"], np.float32),
        outw=np.asarray(inputs["outw"], np.float32),
        rmsw=np.asarray(inputs["rmsw"], np.float32),
        lnw=np.asarray(inputs["lnw"], np.float32),
        lnb=np.asarray(inputs["lnb"], np.float32),
    )
    in_maps, core_meta = [], []
    for i in range(2):
        for dr in range(2):
            for b in range(2):
                in_maps.append(pack_core_inputs(i, dr, b, **args))
                core_meta.append((i, dr, b))
    return in_maps, core_meta


def assemble_outputs(results, core_meta):
    B = 2
    outs = []
    for i in range(2):
        acc = np.zeros((B, C, L_FULL), np.float32)
        for (ii, dr, b), res in zip(core_meta, results):
            if ii != i:
                continue
            p = res["p"]
            if dr == 1:
                p = p[:, ::-1]
            acc[b] += p
        outs.append(acc.reshape(B, C, HH, WW))
    return tuple(outs)


# ------------------------------------------------------------- PJRT executor

class _BassExec:
    def __init__(self, nc, n_cores):
        import jax
        from jax.sharding import Mesh, PartitionSpec
        from jax.experimental.shard_map import shard_map
        from concourse.bass2jax import (_bass_exec_p, install_neuronx_cc_hook,
                                        partition_id_tensor)
        install_neuronx_cc_hook()
        self.jax = jax
        self.n_cores = n_cores
        partition_name = (nc.partition_id_tensor.name
                          if nc.partition_id_tensor else None)
        in_names, out_names, out_avals, zero_outs = [], [], [], []
        for alloc in nc.m.functions[0].allocations:
            if not isinstance(alloc, mybir.MemoryLocationSet):
                continue
            name = alloc.memorylocations[0].name
            if alloc.kind == "ExternalInput":
                if name != partition_name:
                    in_names.append(name)
            elif alloc.kind == "ExternalOutput":
                shape = tuple(alloc.tensor_shape)
                dtype = mybir.dt.np(alloc.dtype)
                out_names.append(name)
                out_avals.append(jax.core.ShapedArray(shape, dtype))
                zero_outs.append(np.zeros(shape, dtype))
        self.in_names, self.out_names = in_names, out_names
        self.out_avals, self.zero_outs = out_avals, zero_outs
        n_params, n_outs = len(in_names), len(out_avals)
        bind_names = in_names + out_names + ([partition_name] if partition_name
                                             else [])

        def _body(*args):
            operands = list(args)
            if partition_name is not None:
                operands.append(partition_id_tensor())
            outs = _bass_exec_p.bind(
                *operands,
                out_avals=tuple(out_avals),
                in_names=tuple(bind_names),
                out_names=tuple(out_names),
                lowering_input_output_aliases=(),
                sim_require_finite=True,
                sim_require_nnan=True,
                nc=nc,
            )
            return tuple(outs)

        devices = jax.devices()[:n_cores]
        self.mesh = Mesh(np.asarray(devices), ("core",))
        in_specs = (PartitionSpec("core"),) * (n_params + n_outs)
        out_specs = (PartitionSpec("core"),) * n_outs
        self.fn = jax.jit(
            shard_map(_body, mesh=self.mesh, in_specs=in_specs,
                      out_specs=out_specs, check_rep=False),
            keep_unused=True)

    def prep(self, in_maps):
        from jax.sharding import NamedSharding, PartitionSpec
        concat_in = [
            np.concatenate([np.asarray(in_maps[c][n])
                            for c in range(self.n_cores)], axis=0)
            for n in self.in_names
        ]
        concat_zero = [
            np.zeros((self.n_cores * z.shape[0], *z.shape[1:]), z.dtype)
            for z in self.zero_outs
        ]
        sh = NamedSharding(self.mesh, PartitionSpec("core"))
        return [self.jax.device_put(a, sh) for a in concat_in + concat_zero]

    def run(self, args):
        outs = self.fn(*args)
        self.jax.block_until_ready(outs)
        return outs

    def results(self, outs):
        res = []
        for c in range(self.n_cores):
            m = {}
            for i, name in enumerate(self.out_names):
                a = np.asarray(outs[i])
                a = a.reshape(self.n_cores, *self.out_avals[i].shape)[c]
                m[name] = a
            res.append(m)
        return res


_CACHE = {}


def _get_exec(pow_dA=False):
    key = f"ex{int(pow_dA)}"
    if key not in _CACHE:
        nc = build_program(pow_dA=pow_dA)
        _CACHE[key] = _BassExec(nc, 8)
    return _CACHE[key]


def kernel(**inputs):
    H = int(inputs.get("H", HH))
    W = int(inputs.get("W", WW))
    assert H == HH and W == WW, (H, W)
    in_maps, core_meta = make_in_maps(inputs)
    ex = _get_exec(pow_dA=False)
    args = ex.prep(in_maps)
    outs = ex.run(args)
    res = ex.results(outs)
    return assemble_outputs(res, core_meta)


# revision 40
# speedup vs baseline: 1.2009x; 1.2009x over previous
"""BiMambaBlock kernel for 8 TRN2 NeuronCores (Bass/Tile via PJRT).

Sharding: 8 cores = (modality i, direction dir, batch b) - each core runs the
full per-sequence pipeline on one channel-shuffled (and, for dir=1, L-flipped)
sequence x_i[b] of shape (96, 9216):
  LayerNorm -> RMSNorm -> fused causal-conv+input-projection -> SiLU
  -> B/C/dt projections -> selective scan (DVE tensor_tensor_scan over
  (d,n)-partition tiles, chunked along L with carried state) ->
  y = (scan + xc*Dp) * silu(z) -> 0.5 * output projection (+ residual on the
  fwd core). Host sums fwd/bwd partials and reshapes.

v3 engine plan (vs the first version):
  - packed scan layout: each scan tile holds 128 partitions = 4 states x
    32 channels (p = a*32+j covers n=4*g_n+a, d=32*g_d+j), so a chunk needs
    24 scan groups instead of 32; dt/dtx are staged to DRAM per chunk and
    replicated into the packed tiles by stride-0 DMA reads, B/C rows
    likewise; the y = sum_n h*C reduction uses three 0/1 reduction
    matrices on the PE with PSUM accumulation across all 24 groups.
  - this removes all per-(n) PE broadcast matmuls and Activation-engine
    PSUM->SBUF copies from the scan inner loop; DMA engines (otherwise
    idle) carry the broadcasts.
  - the B/C/dt projection path runs in bf16 (single PSUM tile + one bf16
    copy per sub-chunk), as do xc/g/dt/dtx/ya/yg; dA stays fp32; dA tiles
    are prefetched one scan-group ahead so Activation bursts don't stall
    the DVE backbone.
  - scan-state carry columns are DVE tensor_copies (SBUF->SBUF), not DMAs;
    the silu "+1" runs on Activation (Identity with bias).
  - bt=dtx*B and the scan run on DVE; h*C runs on Pool (the engine splits
    were tuned against the instruction-cost timeline simulator; Pool
    rejects TensorTensorScanArith and two-tensor TensorScalarPtr on real
    hardware, so its menu is plain tensor_tensor / tensor_scalar).
  - a post-compile pass collapses the alternating exp/ln activation-table
    loads into one load of the natural_log_exp_and_others set.

Self-contained: only needs numpy + jax + the concourse stack at
/opt/trn_rl_repo (present in the execution container).
"""
import sys
for _p in ("/opt/trn_rl_repo",):
    if _p not in sys.path:
        sys.path.insert(0, _p)
import numpy as np
from contextlib import ExitStack

import concourse.bass as bass
import concourse.bacc as bacc
import concourse.tile as tile
from concourse import mybir

F32 = mybir.dt.float32
F32R = mybir.dt.float32r
BF16 = mybir.dt.bfloat16
AF = mybir.ActivationFunctionType
OP = mybir.AluOpType

C, DI, N, R, K = 96, 192, 16, 6, 4
HH = WW = 96
L_FULL = HH * WW     # 9216
EPS = 1e-5

TC = 768             # time chunk
SUB = 384            # psum sub-chunk

# engine-assignment knobs for the scan inner loop: (n*2+hf) pairs with
# idx < HC_DVE_K run the h*C multiply on DVE, the rest on Pool; pairs with
# idx < BT_DVE_K run the dtx*B multiply on DVE, the rest on Pool.
HC_DVE_K = 0
BT_DVE_K = 32


def _fix_act_tables(nc):
    """Replace the alternating exp/ln table loads with a single load of a
    set containing every activation function the program uses."""
    from concourse.hw_specs import get_activation_tables
    used = set()
    for b in nc.main_func.blocks:
        for i in b.instructions:
            if isinstance(i, mybir.InstActivation):
                used.add(i.func)
    tables = get_activation_tables(nc.m.arch)
    target = None
    for idx, (name, funcs) in enumerate(tables.items()):
        if used <= funcs:
            target = idx
            break
    if target is None:
        return  # no single covering set; leave the compiler's placement
    for b in nc.main_func.blocks:
        keep_done = False
        new_insts = []
        for i in b.instructions:
            if isinstance(i, mybir.InstLoadActFuncSet):
                si = i.sync_info
                clean = si is None or (len(si.on_wait) == 0
                                       and len(si.on_update) == 0)
                if not keep_done or not clean:
                    i.act_func_set_id = target
                    keep_done = True
                    new_insts.append(i)
                # else: drop redundant load
            else:
                new_insts.append(i)
        b.instructions[:] = new_insts


def build_program(L=L_FULL, Tc=TC, pow_dA=False):
    NCHUNK = L // Tc
    NSUB = Tc // SUB
    TC3 = Tc + 3
    nc = bacc.Bacc("TRN2", target_bir_lowering=False, debug=False)

    x_in = nc.dram_tensor("x", [C, L], F32, kind="ExternalInput")
    wIN = nc.dram_tensor("wIN", [C, K * DI], F32, kind="ExternalInput")
    wZ = nc.dram_tensor("wZ", [C, DI], F32, kind="ExternalInput")
    wXPb = nc.dram_tensor("wXPb", [C, 2 * 38], BF16, kind="ExternalInput")
    wDTb = nc.dram_tensor("wDTb", [R, DI], BF16, kind="ExternalInput")
    wA2 = nc.dram_tensor("wA2", [128, 24], F32, kind="ExternalInput")
    wOUTb = nc.dram_tensor("wOUTb", [C, 2 * C], BF16, kind="ExternalInput")
    vec2 = nc.dram_tensor("vec2", [C, 8], F32, kind="ExternalInput")
    vec1 = nc.dram_tensor("vec1", [C, 3], F32, kind="ExternalInput")
    gate_in = nc.dram_tensor("gate", [1, 1], F32, kind="ExternalInput")
    red_in = nc.dram_tensor("red", [128, 3 * C], BF16, kind="ExternalInput")

    # ping-pong staging for the per-chunk B/C and dt/dtx rows
    # (replication/broadcast-DMA sources)
    bc_d = [nc.dram_tensor(f"bcrows{p}", [2 * N, Tc], BF16, kind="Internal")
            for p in range(2)]
    dt_d = [nc.dram_tensor(f"dtrows{p}", [2 * C, Tc], BF16, kind="Internal")
            for p in range(2)]
    dx_d = [nc.dram_tensor(f"dxrows{p}", [2 * C, Tc], BF16, kind="Internal")
            for p in range(2)]

    p_out = nc.dram_tensor("p", [C, L], F32, kind="ExternalOutput")

    with ExitStack() as ctx:
        tc = ctx.enter_context(tile.TileContext(nc))
        wp = ctx.enter_context(tc.tile_pool(name="wts", bufs=1))
        px = ctx.enter_context(tc.tile_pool(name="px", bufs=3))
        ph = ctx.enter_context(tc.tile_pool(name="ph", bufs=3))
        pt0 = ctx.enter_context(tc.tile_pool(name="pt0", bufs=2))
        pt1 = ctx.enter_context(tc.tile_pool(name="pt1", bufs=2))
        psp = ctx.enter_context(tc.tile_pool(name="psp", bufs=2))
        prow = ctx.enter_context(tc.tile_pool(name="prow", bufs=2))
        pxc = ctx.enter_context(tc.tile_pool(name="pxc", bufs=2))
        pg = ctx.enter_context(tc.tile_pool(name="pg", bufs=2))
        pdbl = ctx.enter_context(tc.tile_pool(name="pdbl", bufs=2))
        pdt = ctx.enter_context(tc.tile_pool(name="pdt", bufs=2))
        pdtx = ctx.enter_context(tc.tile_pool(name="pdtx", bufs=2))
        pbb = ctx.enter_context(tc.tile_pool(name="pbb", bufs=2))
        psc = ctx.enter_context(tc.tile_pool(name="psc", bufs=2))
        phh = ctx.enter_context(tc.tile_pool(name="phh", bufs=2))
        phc = ctx.enter_context(tc.tile_pool(name="phc", bufs=2))
        pst = ctx.enter_context(tc.tile_pool(name="pst", bufs=1))
        ptail = ctx.enter_context(tc.tile_pool(name="ptail", bufs=2))
        ppr = ctx.enter_context(tc.tile_pool(name="ppr", bufs=1))

        qh = ctx.enter_context(tc.tile_pool(name="qh", bufs=2, space="PSUM"))
        qdbl = ctx.enter_context(tc.tile_pool(name="qdbl", bufs=2,
                                              space="PSUM"))
        qy = ctx.enter_context(tc.tile_pool(name="qy", bufs=1, space="PSUM"))

        w_in = wp.tile([C, K * DI], F32); nc.sync.dma_start(w_in[:], wIN[:])
        w_z = wp.tile([C, DI], F32); nc.sync.dma_start(w_z[:], wZ[:])
        w_xpb = wp.tile([C, 2 * 38], BF16); nc.sync.dma_start(w_xpb[:],
                                                             wXPb[:])
        w_dtb = wp.tile([R, DI], BF16); nc.sync.dma_start(w_dtb[:], wDTb[:])
        w_a2 = wp.tile([128, 24], F32); nc.sync.dma_start(w_a2[:], wA2[:])
        w_outb = wp.tile([C, 2 * C], BF16); nc.sync.dma_start(w_outb[:],
                                                             wOUTb[:])
        v2 = wp.tile([C, 8], F32); nc.sync.dma_start(v2[:], vec2[:])
        v1 = wp.tile([C, 3], F32); nc.sync.dma_start(v1[:], vec1[:])
        gt = wp.tile([1, 1], F32); nc.sync.dma_start(gt[:], gate_in[:])

        # fp32r-rounded copies of the stationary matmul operands
        w_inr = wp.tile([C, K * DI], F32R); nc.scalar.copy(w_inr[:], w_in[:])
        w_zr = wp.tile([C, DI], F32R); nc.scalar.copy(w_zr[:], w_z[:])
        red_b = wp.tile([128, 3 * C], BF16)
        nc.sync.dma_start(red_b[:], red_in[:])

        ones_col = wp.tile([C, 1], F32); nc.vector.memset(ones_col[:], 1.0)
        ones_col_r = wp.tile([C, 1], F32R); nc.scalar.copy(ones_col_r[:],
                                                          ones_col[:])
        ones_row = wp.tile([1, C], F32); nc.vector.memset(ones_row[:], 1.0)
        ones_row_r = wp.tile([1, C], F32R); nc.scalar.copy(ones_row_r[:],
                                                          ones_row[:])
        epsc = wp.tile([1, 1], F32); nc.vector.memset(epsc[:], EPS)
        gate_c = wp.tile([C, 1], F32)
        qg = qh.tile([C, 1], F32, tag="a")
        nc.tensor.matmul(qg[:], ones_row[:], gt[:])
        nc.scalar.copy(gate_c[:], qg[:])

        # probes: absorb cross-engine waits (TensorScalarPtr ops: 1 wait slot)
        prv = ppr.tile([1, 8], F32)
        pra = ppr.tile([1, 8], F32)
        nc.vector.tensor_copy(prv[:, 0:1], v1[:1, 0:1])
        nc.vector.tensor_copy(prv[:, 1:2], v2[:1, 0:1])
        nc.vector.tensor_copy(prv[:, 2:3], gate_c[:1, 0:1])
        nc.scalar.copy(pra[:, 0:1], w_a2[:1, 0:1])
        nc.scalar.copy(pra[:, 1:2], v1[:1, 0:1])
        nc.scalar.copy(pra[:, 2:3], v2[:1, 0:1])

        st = pst.tile([128, 24], BF16)
        nc.vector.memset(st[:], 0.0)
        zero3 = wp.tile([C, 3], F32); nc.vector.memset(zero3[:], 0.0)

        def make_prologue(ci, chunks):
            """Emit-later closures for chunk ci's pre-scan pipeline. Each
            stage is emitted interleaved with the previous chunk's scan
            iterations so the in-order engine queues overlap them."""
            S = {"t0": ci * Tc}
            t0 = S["t0"]

            def s_dma():
                S["xt"] = px.tile([C, Tc], F32, tag="xt", name="xt")
                nc.sync.dma_start(S["xt"][:], x_in[:, t0:t0 + Tc])

            def s_sq():
                S["sq"] = pt0.tile([C, Tc], F32R, tag="sql", name="sq")
                nc.scalar.activation(S["sq"][:], S["xt"][:], AF.Square)

            def s_stats():
                S["m_"] = prow.tile([1, Tc], F32R, tag="m", bufs=2, name="m_")
                S["var_"] = prow.tile([1, Tc], F32, tag="var", bufs=1,
                                      name="var_")
                mm_ = prow.tile([1, Tc], F32, tag="mm", bufs=1, name="mm_")
                for si in range(NSUB):
                    o = si * SUB
                    s1 = qh.tile([1, SUB], F32, tag="a", name="s1")
                    nc.tensor.matmul(s1[:], ones_col[:], S["xt"][:, o:o + SUB])
                    nc.vector.tensor_scalar_mul(
                        S["m_"][:, o:o + SUB], s1[:], 1.0 / C)
                    s2 = qh.tile([1, SUB], F32, tag="a", name="s2")
                    nc.tensor.matmul(s2[:], ones_col_r[:],
                                     S["sq"][:, o:o + SUB])
                    nc.vector.tensor_tensor(
                        mm_[:, o:o + SUB], S["m_"][:, o:o + SUB].bitcast(F32),
                        S["m_"][:, o:o + SUB].bitcast(F32), op=OP.mult)
                    nc.vector.scalar_tensor_tensor(
                        S["var_"][:, o:o + SUB], s2[:], 1.0 / C,
                        mm_[:, o:o + SUB], op0=OP.mult, op1=OP.subtract)

            def s_rstd():
                lnv = prow.tile([1, Tc], F32, tag="lnv", bufs=1, name="lnv")
                S["rstd"] = prow.tile([1, Tc], F32R, tag="rstd", bufs=2,
                                      name="rstd")
                nc.scalar.activation(lnv[:], S["var_"][:], AF.Ln,
                                     bias=epsc[:, 0:1])
                nc.scalar.activation(S["rstd"][:], lnv[:], AF.Exp, scale=-0.5)

            def s_xn():
                S["xn"] = pt1.tile([C, Tc], F32, tag="xn", name="xn")
                for si in range(NSUB):
                    o = si * SUB
                    mb = qh.tile([C, SUB], F32, tag="a", name="mb")
                    nc.tensor.matmul(mb[:], ones_row_r[0:1, :],
                                     S["m_"][:, o:o + SUB])
                    nc.vector.tensor_tensor(S["xn"][:, o:o + SUB],
                                            S["xt"][:, o:o + SUB],
                                            mb[:], op=OP.subtract)
                    rb = qh.tile([C, SUB], F32, tag="a", name="rb")
                    nc.tensor.matmul(rb[:], ones_row_r[0:1, :],
                                     S["rstd"][:, o:o + SUB])
                    nc.vector.tensor_tensor(S["xn"][:, o:o + SUB],
                                            S["xn"][:, o:o + SUB],
                                            rb[:], op=OP.mult)

            def s_ln():
                S["ln_t"] = pt1.tile([C, Tc], F32, tag="ln", name="ln_t")
                nc.scalar.activation(S["ln_t"][:], S["xn"][:], AF.Identity,
                                     bias=v1[:, 1:2], scale=v1[:, 0:1])
                S["lsq"] = pt0.tile([C, Tc], F32R, tag="sql", name="lsq")
                nc.scalar.activation(S["lsq"][:], S["ln_t"][:], AF.Square)

            def s_rr():
                lnr = prow.tile([1, Tc], F32, tag="lnr", bufs=1, name="lnr")
                S["rr"] = prow.tile([1, Tc], F32R, tag="rr", bufs=2, name="rr")
                for si in range(NSUB):
                    o = si * SUB
                    s3 = qh.tile([1, SUB], F32, tag="a", name="s3")
                    nc.tensor.matmul(s3[:], ones_col_r[:],
                                     S["lsq"][:, o:o + SUB])
                    nc.scalar.activation(lnr[:, o:o + SUB], s3[:],
                                         AF.Ln, scale=1.0 / C,
                                         bias=epsc[:, 0:1])
                    nc.scalar.activation(S["rr"][:, o:o + SUB],
                                         lnr[:, o:o + SUB], AF.Exp, scale=-0.5)

            def s_h():
                h_t = ph.tile([C, TC3], F32R, tag="h", name="h_t")
                S["h_t"] = h_t
                if ci == 0:
                    nc.scalar.copy(h_t[:, 0:3], zero3[:])
                else:
                    hp = chunks[ci - 1]["h_t"]
                    nc.vector.tensor_copy(h_t[:, 0:3],
                                          hp[:, Tc:Tc + 3].bitcast(F32))
                nc.vector.tensor_copy(prv[:, 3:4], S["ln_t"][:1, 0:1])
                for si in range(NSUB):
                    o = si * SUB
                    rrb = qh.tile([C, SUB], F32, tag="a", name="rrb")
                    nc.tensor.matmul(rrb[:], ones_row_r[0:1, :],
                                     S["rr"][:, o:o + SUB])
                    nc.vector.scalar_tensor_tensor(
                        h_t[:, 3 + o:3 + o + SUB], S["ln_t"][:, o:o + SUB],
                        v1[:, 2:3], rrb[:], op0=OP.mult, op1=OP.mult)

            def make_conv(hf, si):
                def s_conv():
                    if "xc_h" not in S:
                        S["xc_h"] = [None, None]
                        S["g_h"] = [None, None]
                    if S["xc_h"][hf] is None:
                        S["xc_h"][hf] = pxc.tile([C, Tc], BF16, tag=f"xc{hf}",
                                                 name=f"xc{hf}")
                        S["g_h"][hf] = pg.tile([C, Tc], BF16, tag=f"g{hf}",
                                               name=f"g{hf}")
                    xc = S["xc_h"][hf]
                    g = S["g_h"][hf]
                    h_t = S["h_t"]
                    if True:
                        o = si * SUB
                        ps = qh.tile([C, SUB], F32, tag="a", name="psc1")
                        for k in range(K):
                            nc.tensor.matmul(
                                ps[:],
                                w_inr[:, k * DI + hf * C:k * DI + hf * C + C],
                                h_t[:, o + k:o + k + SUB],
                                start=(k == 0), stop=(k == K - 1))
                        # silu(p+cb) = (p+cb) / (1+exp(-(p+cb)))
                        e1 = psp.tile([C, SUB], F32, tag="sg1", name="e1")
                        nc.scalar.activation(e1[:], ps[:], AF.Exp, scale=-1.0,
                                             bias=v2[:, 6 + hf:7 + hf])
                        f1 = psp.tile([C, SUB], F32, tag="sg3", name="f1")
                        nc.scalar.activation(f1[:], e1[:], AF.Identity,
                                             bias=1.0)
                        r1 = psp.tile([C, SUB], F32, tag="sg2", name="r1")
                        nc.vector.reciprocal_approx_fast(r1[:], f1[:])
                        nc.vector.scalar_tensor_tensor(
                            xc[:, o:o + SUB], ps[:], v2[:, hf:hf + 1], r1[:],
                            op0=OP.add, op1=OP.mult)
                        ps2 = qh.tile([C, SUB], F32, tag="a", name="psc2")
                        nc.tensor.matmul(ps2[:], w_zr[:, hf * C:hf * C + C],
                                         h_t[:, o + 3:o + 3 + SUB])
                        e2 = psp.tile([C, SUB], F32, tag="sg1", name="e2")
                        nc.scalar.activation(e2[:], ps2[:], AF.Exp, scale=-1.0)
                        f2 = psp.tile([C, SUB], F32, tag="sg3", name="f2")
                        nc.scalar.activation(f2[:], e2[:], AF.Identity,
                                             bias=1.0)
                        r2 = psp.tile([C, SUB], F32, tag="sg2", name="r2")
                        nc.vector.reciprocal_approx_fast(r2[:], f2[:])
                        nc.vector.tensor_tensor(g[:, o:o + SUB], ps2[:], r2[:],
                                                op=OP.mult)
                return s_conv

            def make_dbl(si):
                # B/C/dt projections in one PSUM tile; bf16 copy. Matmul
                # outputs must start at partition 0 or 32 and match the
                # lhsT base, so: rows [0:32) = dt-proj + a redundant B/C
                # prefix (keeps every copied row defined), rows [32:64) =
                # B/C. dt-proj = dbc[0:R], B = dbc[32:48], C = dbc[48:64].
                def s_dbl():
                    if "dbc" not in S:
                        S["dbc"] = pdbl.tile([64, Tc], BF16, tag="dbc",
                                             name="dbc")
                    o = si * SUB
                    ps = qdbl.tile([64, SUB], F32, tag="d", name="psdbl")
                    for lo, src_lo in ((0, 0), (32, R)):
                        for hf in range(2):
                            nc.tensor.matmul(
                                ps[lo:lo + 32, :],
                                w_xpb[:, hf * 38 + src_lo:
                                      hf * 38 + src_lo + 32],
                                S["xc_h"][hf][:, o:o + SUB],
                                start=(hf == 0), stop=(hf == 1))
                    nc.scalar.copy(S["dbc"][:, o:o + SUB], ps[:])
                    if si == NSUB - 1:
                        # stage B/C rows to DRAM for the broadcast reads
                        nc.sync.dma_start(bc_d[ci % 2][:, :],
                                          S["dbc"][32:64, :])
                return s_dbl

            def make_dt(hf):
                def s_dt():
                    if "dt_h" not in S:
                        S["dt_h"] = [None, None]
                    S["dt_h"][hf] = pdt.tile([C, Tc], BF16, tag=f"dt{hf}",
                                             name=f"dt{hf}")
                    for si in range(NSUB):
                        o = si * SUB
                        ps = qh.tile([C, SUB], F32, tag="a", name="psdt")
                        nc.tensor.matmul(ps[:], w_dtb[:, hf * C:hf * C + C],
                                         S["dbc"][0:R, o:o + SUB])
                        # softplus: dt projections sit near dtb ~ -4, so the
                        # direct ln(1+exp(v)) form cannot overflow
                        ex = psp.tile([C, SUB], F32, tag="spe", name="ex")
                        nc.scalar.activation(ex[:], ps[:], AF.Exp,
                                             bias=v2[:, 2 + hf:3 + hf])
                        nc.scalar.activation(S["dt_h"][hf][:, o:o + SUB],
                                             ex[:], AF.Ln, bias=1.0)
                    nc.sync.dma_start(dt_d[ci % 2][hf * C:(hf + 1) * C, :],
                                      S["dt_h"][hf][:])
                return s_dt

            def make_dtx(hf):
                def s_dtx():
                    dx = pdtx.tile([C, Tc], BF16, tag=f"dtx{hf}",
                                   name=f"dtx{hf}")
                    nc.vector.tensor_tensor(dx[:], S["dt_h"][hf][:],
                                            S["xc_h"][hf][:], op=OP.mult)
                    nc.sync.dma_start(dx_d[ci % 2][hf * C:(hf + 1) * C, :],
                                      dx[:])
                return s_dtx

            def _rep_from(dram, r0):
                # (128,Tc) <- rows [r0, r0+32) of `dram`, each row at the 4
                # partitions a*32+j (a = n-subindex, j = d-subindex)
                sl = dram[r0:r0 + 32, :]
                return bass.AP(tensor=sl.tensor, offset=sl.offset,
                               ap=[[0, 4]] + [list(a) for a in sl.ap])

            def make_rep(hf):
                def s_rep():
                    if "dtp" not in S:
                        S["dtp"] = [[None] * 3, [None] * 3]
                        S["dxp"] = [[None] * 3, [None] * 3]
                    for g_d in range(3):
                        tp = pdt.tile([128, Tc], BF16, tag=f"dtp{hf}{g_d}",
                                      name=f"dtp{hf}{g_d}")
                        nc.sync.dma_start(
                            tp[:], _rep_from(dt_d[ci % 2], hf * C + 32 * g_d))
                        S["dtp"][hf][g_d] = tp
                        xp = pdtx.tile([128, Tc], BF16, tag=f"dxp{hf}{g_d}",
                                       name=f"dxp{hf}{g_d}")
                        nc.sync.dma_start(
                            xp[:], _rep_from(dx_d[ci % 2], hf * C + 32 * g_d))
                        S["dxp"][hf][g_d] = xp
                return s_rep

            early = [s_dma, s_sq, s_stats, s_rstd, s_xn, s_ln, s_rr, s_h]
            late = [make_conv(0, 0), make_conv(0, 1), make_conv(1, 0),
                    make_conv(1, 1), make_dbl(0), make_dbl(1),
                    make_dt(0), make_dt(1), make_dtx(0), make_dtx(1),
                    make_rep(0), make_rep(1)]
            return S, early, late

        def emit_scan_and_tail(S, ci, nxt_stages, nxt_S=None):
            t0 = S["t0"]
            xt, xc_h, g_h = S["xt"], S["xc_h"], S["g_h"]
            dtp, dxp = S["dtp"], S["dxp"]
            yps = [[qy.tile([C, SUB], F32, tag=f"y{hf}_{si}",
                            name=f"y{hf}_{si}")
                    for si in range(NSUB)] for hf in range(2)]

            def emit_bc(g_n):
                # (128,Tc) b/c tiles: rows 4*g_n+a at partitions a*32+j
                bb = pbb.tile([128, Tc], BF16, tag="bb", name="bb", bufs=4)
                cb = pbb.tile([128, Tc], BF16, tag="cb", name="cb", bufs=4)
                src_ = bc_d[ci % 2]
                for dst, r0 in ((bb, 4 * g_n), (cb, N + 4 * g_n)):
                    sl = src_[r0:r0 + 4, :]
                    ap = [list(sl.ap[0]), [0, 32], list(sl.ap[1])]
                    nc.sync.dma_start(dst[:], bass.AP(
                        tensor=sl.tensor, offset=sl.offset, ap=ap))
                return bb, cb

            iters = [(g_n, hf, g_d) for g_n in range(4) for hf in range(2)
                     for g_d in range(3)]

            def emit_dA(it):
                g_n, hf, g_d = iters[it]
                dA = psc.tile([128, Tc], F32, tag="dA", name="dA", bufs=4)
                nc.scalar.activation(dA[:], dtp[hf][g_d][:], AF.Exp,
                                     scale=w_a2[:, it:it + 1])
                return dA

            S["emit_bc"] = emit_bc
            S["emit_dA"] = emit_dA
            bcq = [S.pop("bc0", None) or emit_bc(0), emit_bc(1)]
            pre = S.pop("dA01", None)
            dAq = pre if pre else [emit_dA(0), emit_dA(1)]
            dAq.append(emit_dA(2))
            for it, (g_n, hf, g_d) in enumerate(iters):
                if g_d == 0 and hf == 0:
                    bb, cb = bcq.pop(0)
                    if g_n + 2 < 4:
                        bcq.append(emit_bc(g_n + 2))
                dA = dAq.pop(0)
                if it + 3 < 24:
                    dAq.append(emit_dA(it + 3))
                bt = psc.tile([128, Tc], BF16, tag="bt", name="bt", bufs=4)
                nc.vector.tensor_tensor(bt[:], dxp[hf][g_d][:], bb[:],
                                        op=OP.mult)
                ht = phh.tile([128, Tc], BF16, tag="ht", name="ht", bufs=4)
                nc.vector.tensor_tensor_scan(
                    ht[:], dA[:], bt[:], st[:, it:it + 1],
                    op0=OP.mult, op1=OP.add)
                nc.vector.tensor_copy(st[:, it:it + 1], ht[:, Tc - 1:Tc])
                hc = phc.tile([128, Tc], BF16, tag="hc", name="hc", bufs=4)
                nc.gpsimd.tensor_tensor(hc[:], ht[:], cb[:], op=OP.mult)
                first = (g_n == 0 and g_d == 0)
                last = (g_n == 3 and g_d == 2)
                for si in range(NSUB):
                    o = si * SUB
                    nc.tensor.matmul(yps[hf][si][:],
                                     red_b[:, g_d * C:(g_d + 1) * C],
                                     hc[:, o:o + SUB],
                                     start=first, stop=last,
                                     skip_group_check=True)
                if nxt_stages:
                    nxt_stages.pop(0)()

            while nxt_stages:
                nxt_stages.pop(0)()
            yg_h = []
            for hf in range(2):
                ya = ptail.tile([C, Tc], BF16, tag=f"ya{hf}", name=f"ya{hf}",
                                bufs=1)
                for si in range(NSUB):
                    o = si * SUB
                    nc.vector.scalar_tensor_tensor(
                        ya[:, o:o + SUB], xc_h[hf][:, o:o + SUB],
                        v2[:, 4 + hf:5 + hf], yps[hf][si][:],
                        op0=OP.mult, op1=OP.add)
                yg = ptail.tile([C, Tc], BF16, tag=f"yg{hf}", name=f"yg{hf}",
                                bufs=1)
                nc.gpsimd.tensor_tensor(yg[:], ya[:], g_h[hf][:], op=OP.mult)
                yg_h.append(yg)
            for si in range(NSUB):
                o = si * SUB
                pso = qh.tile([C, SUB], F32, tag="a", name="pso")
                for hf in range(2):
                    nc.tensor.matmul(pso[:], w_outb[:, hf * C:hf * C + C],
                                     yg_h[hf][:, o:o + SUB],
                                     start=(hf == 0), stop=(hf == 1))
                ot = ptail.tile([C, SUB], F32, tag="ot", name="ot")
                nc.vector.scalar_tensor_tensor(
                    ot[:], xt[:, o:o + SUB], gate_c[:, 0:1], pso[:],
                    op0=OP.mult, op1=OP.add)
                nc.sync.dma_start(p_out[:, t0 + o:t0 + o + SUB], ot[:])

        chunks = {}
        S0, early0, late0 = make_prologue(0, chunks)
        chunks[0] = S0
        for f in early0 + late0:
            f()
        pend_late = {}
        if NCHUNK > 1:
            S1, early1, late1 = make_prologue(1, chunks)
            chunks[1] = S1
            for f in early1:
                f()
            pend_late[1] = late1
        for ci in range(NCHUNK):
            stages = list(pend_late.pop(ci + 1, []))
            if ci + 2 < NCHUNK:
                S2, early2, late2 = make_prologue(ci + 2, chunks)
                chunks[ci + 2] = S2
                stages += early2
                pend_late[ci + 2] = late2
            emit_scan_and_tail(chunks[ci], ci, stages,
                               chunks.get(ci + 1))
            chunks.pop(ci - 1, None)

    nc.compile()
    _fix_act_tables(nc)
    return nc


# ---------------------------------------------------------------- host side

def shuffle_channels(x):
    c = x.shape[0]
    return x.reshape(2, c // 2, -1).transpose(1, 0, 2).reshape(c, -1)


def _bf16():
    try:
        import ml_dtypes
        return ml_dtypes.bfloat16
    except Exception:
        import jax.numpy as _jnp
        return _jnp.bfloat16


def pack_core_inputs(i, dr, b, x1, x2, inw, convw, convb, xpw, dtw, dtb,
                     Alog, Dp, outw, rmsw, lnw, lnb):
    xs = x1 if i == 0 else x2
    x = shuffle_channels(np.asarray(xs[b], np.float32))
    if dr == 1:
        x = x[:, ::-1]
    x = np.ascontiguousarray(x)

    inw_i = np.asarray(inw[i], np.float32)
    cw = np.asarray(convw[i, dr], np.float32)
    cb = np.asarray(convb[i, dr], np.float32)
    xp = np.asarray(xpw[i, dr], np.float32)
    dw = np.asarray(dtw[i, dr], np.float32)
    db = np.asarray(dtb[i, dr], np.float32)
    Av = -np.exp(np.asarray(Alog[i, dr], np.float32))
    Dv = np.asarray(Dp[i, dr], np.float32)
    ow = np.asarray(outw[i], np.float32)

    wIN = np.empty((C, K * DI), np.float32)
    inw_x = inw_i[:DI]
    for k in range(K):
        wIN[:, k * DI:(k + 1) * DI] = (cw[:, k][:, None] * inw_x).T
    wZ = np.ascontiguousarray(inw_i[DI:].T)
    bf16 = _bf16()
    wXP = np.empty((C, 2 * 38), np.float32)
    for hf in range(2):
        wXP[:, hf * 38:(hf + 1) * 38] = xp[:, hf * C:(hf + 1) * C].T
    wXPb = wXP.astype(bf16)
    wDTb = np.ascontiguousarray(dw.T).astype(bf16)
    # packed-layout A scales: column it=(g_n*6+hf*3+g_d) holds, at
    # partition p=a*32+j, the A value for n=4*g_n+a, d=32*g_d+j
    wA2 = np.empty((128, 24), np.float32)
    for g_n in range(4):
        for hf in range(2):
            for g_d in range(3):
                it = g_n * 6 + hf * 3 + g_d
                for a in range(4):
                    for j in range(32):
                        wA2[a * 32 + j, it] = Av[hf * C + 32 * g_d + j,
                                                 4 * g_n + a]
    wOUT = np.empty((C, 2 * C), np.float32)
    for hf in range(2):
        wOUT[:, hf * C:(hf + 1) * C] = 0.5 * ow[:, hf * C:(hf + 1) * C].T
    wOUTb = wOUT.astype(bf16)
    vec2 = np.ascontiguousarray(
        np.stack([cb[:C], cb[C:], db[:C], db[C:], Dv[:C], Dv[C:],
                  -cb[:C], -cb[C:]], axis=1), dtype=np.float32)
    vec1 = np.ascontiguousarray(
        np.stack([np.asarray(lnw[i], np.float32),
                  np.asarray(lnb[i], np.float32),
                  np.asarray(rmsw[i], np.float32)], axis=1), dtype=np.float32)
    gate = np.array([[1.0 if dr == 0 else 0.0]], np.float32)
    red = np.zeros((128, 3 * C), np.float32)
    for g_d in range(3):
        for a in range(4):
            for j in range(32):
                red[a * 32 + j, g_d * C + 32 * g_d + j] = 1.0
    red = red.astype(bf16)
    return {
        "x": x, "wIN": wIN, "wZ": wZ, "wXPb": wXPb, "wDTb": wDTb,
        "wA2": wA2, "wOUTb": wOUTb, "vec2": vec2, "vec1": vec1,
        "gate": gate, "red": red,
    }


def make_in_maps(inputs):
    args = dict(
        x1=np.asarray(inputs["x1"], np.float32),
        x2=np.asarray(inputs["x2"], np.float32),
        inw=np.asarray(inputs["inw"], np.float32),
        convw=np.asarray(inputs["convw"], np.float32),
        convb=np.asarray(inputs["convb"], np.float32),
        xpw=np.asarray(inputs["xpw"], np.float32),
        dtw=np.asarray(inputs["dtw"], np.float32),
        dtb=np.asarray(inputs["dtb"], np.float32),
        Alog=np.asarray(inputs["Alog"], np.float32),
        Dp=np.asarray(inputs["Dp"], np.float32),
        outw=np.asarray(inputs["outw"], np.float32),
        rmsw=np.asarray(inputs["rmsw"], np.float32),
        lnw=np.asarray(inputs["lnw"], np.float32),
        lnb=np.asarray(inputs["lnb"], np.float32),
    )
    in_maps, core_meta = [], []
    for i in range(2):
        for dr in range(2):
            for b in range(2):
                in_maps.append(pack_core_inputs(i, dr, b, **args))
                core_meta.append((i, dr, b))
    return in_maps, core_meta


def assemble_outputs(results, core_meta):
    B = 2
    outs = []
    for i in range(2):
        acc = np.zeros((B, C, L_FULL), np.float32)
        for (ii, dr, b), res in zip(core_meta, results):
            if ii != i:
                continue
            p = res["p"]
            if dr == 1:
                p = p[:, ::-1]
            acc[b] += p
        outs.append(acc.reshape(B, C, HH, WW))
    return tuple(outs)


# ------------------------------------------------------------- PJRT executor

class _BassExec:
    def __init__(self, nc, n_cores):
        import jax
        from jax.sharding import Mesh, PartitionSpec
        from jax.experimental.shard_map import shard_map
        from concourse.bass2jax import (_bass_exec_p, install_neuronx_cc_hook,
                                        partition_id_tensor)
        install_neuronx_cc_hook()
        self.jax = jax
        self.n_cores = n_cores
        partition_name = (nc.partition_id_tensor.name
                          if nc.partition_id_tensor else None)
        in_names, out_names, out_avals, zero_outs = [], [], [], []
        for alloc in nc.m.functions[0].allocations:
            if not isinstance(alloc, mybir.MemoryLocationSet):
                continue
            name = alloc.memorylocations[0].name
            if alloc.kind == "ExternalInput":
                if name != partition_name:
                    in_names.append(name)
            elif alloc.kind == "ExternalOutput":
                shape = tuple(alloc.tensor_shape)
                dtype = mybir.dt.np(alloc.dtype)
                out_names.append(name)
                out_avals.append(jax.core.ShapedArray(shape, dtype))
                zero_outs.append(np.zeros(shape, dtype))
        self.in_names, self.out_names = in_names, out_names
        self.out_avals, self.zero_outs = out_avals, zero_outs
        n_params, n_outs = len(in_names), len(out_avals)
        bind_names = in_names + out_names + ([partition_name] if partition_name
                                             else [])

        def _body(*args):
            operands = list(args)
            if partition_name is not None:
                operands.append(partition_id_tensor())
            outs = _bass_exec_p.bind(
                *operands,
                out_avals=tuple(out_avals),
                in_names=tuple(bind_names),
                out_names=tuple(out_names),
                lowering_input_output_aliases=(),
                sim_require_finite=True,
                sim_require_nnan=True,
                nc=nc,
            )
            return tuple(outs)

        devices = jax.devices()[:n_cores]
        self.mesh = Mesh(np.asarray(devices), ("core",))
        in_specs = (PartitionSpec("core"),) * (n_params + n_outs)
        out_specs = (PartitionSpec("core"),) * n_outs
        self.fn = jax.jit(
            shard_map(_body, mesh=self.mesh, in_specs=in_specs,
                      out_specs=out_specs, check_rep=False),
            keep_unused=True)

    def prep(self, in_maps):
        from jax.sharding import NamedSharding, PartitionSpec
        concat_in = [
            np.concatenate([np.asarray(in_maps[c][n])
                            for c in range(self.n_cores)], axis=0)
            for n in self.in_names
        ]
        concat_zero = [
            np.zeros((self.n_cores * z.shape[0], *z.shape[1:]), z.dtype)
            for z in self.zero_outs
        ]
        sh = NamedSharding(self.mesh, PartitionSpec("core"))
        return [self.jax.device_put(a, sh) for a in concat_in + concat_zero]

    def run(self, args):
        outs = self.fn(*args)
        self.jax.block_until_ready(outs)
        return outs

    def results(self, outs):
        res = []
        for c in range(self.n_cores):
            m = {}
            for i, name in enumerate(self.out_names):
                a = np.asarray(outs[i])
                a = a.reshape(self.n_cores, *self.out_avals[i].shape)[c]
                m[name] = a
            res.append(m)
        return res


_CACHE = {}


def _get_exec(pow_dA=False):
    key = f"ex{int(pow_dA)}"
    if key not in _CACHE:
        nc = build_program(pow_dA=pow_dA)
        _CACHE[key] = _BassExec(nc, 8)
    return _CACHE[key]


def kernel(**inputs):
    H = int(inputs.get("H", HH))
    W = int(inputs.get("W", WW))
    assert H == HH and W == WW, (H, W)
    in_maps, core_meta = make_in_maps(inputs)
    ex = _get_exec(pow_dA=False)
    args = ex.prep(in_maps)
    outs = ex.run(args)
    res = ex.results(outs)
    return assemble_outputs(res, core_meta)


# revision 42
# speedup vs baseline: 1.2685x; 1.0563x over previous
"""BiMambaBlock kernel for 8 TRN2 NeuronCores (Bass/Tile via PJRT).

Sharding: 8 cores = (modality i, direction dir, batch b) - each core runs the
full per-sequence pipeline on one channel-shuffled (and, for dir=1, L-flipped)
sequence x_i[b] of shape (96, 9216):
  LayerNorm -> RMSNorm -> fused causal-conv+input-projection -> SiLU
  -> B/C/dt projections -> selective scan (DVE tensor_tensor_scan over
  (d,n)-partition tiles, chunked along L with carried state) ->
  y = (scan + xc*Dp) * silu(z) -> 0.5 * output projection (+ residual on the
  fwd core). Host sums fwd/bwd partials and reshapes.

v3 engine plan (vs the first version):
  - packed scan layout: each scan tile holds 128 partitions = 4 states x
    32 channels (p = a*32+j covers n=4*g_n+a, d=32*g_d+j), so a chunk needs
    24 scan groups instead of 32; dt/dtx are staged to DRAM per chunk and
    replicated into the packed tiles by stride-0 DMA reads, B/C rows
    likewise; the y = sum_n h*C reduction uses three 0/1 reduction
    matrices on the PE with PSUM accumulation across all 24 groups.
  - this removes all per-(n) PE broadcast matmuls and Activation-engine
    PSUM->SBUF copies from the scan inner loop; DMA engines (otherwise
    idle) carry the broadcasts.
  - the B/C/dt projection path runs in bf16 (single PSUM tile + one bf16
    copy per sub-chunk), as do xc/g/dt/dtx/ya/yg; dA stays fp32; dA tiles
    are prefetched one scan-group ahead so Activation bursts don't stall
    the DVE backbone.
  - scan-state carry columns are DVE tensor_copies (SBUF->SBUF), not DMAs;
    the silu "+1" runs on Activation (Identity with bias).
  - bt=dtx*B and the scan run on DVE; h*C runs on Pool (the engine splits
    were tuned against the instruction-cost timeline simulator; Pool
    rejects TensorTensorScanArith and two-tensor TensorScalarPtr on real
    hardware, so its menu is plain tensor_tensor / tensor_scalar).
  - a post-compile pass collapses the alternating exp/ln activation-table
    loads into one load of the natural_log_exp_and_others set.

Self-contained: only needs numpy + jax + the concourse stack at
/opt/trn_rl_repo (present in the execution container).
"""
import sys
for _p in ("/opt/trn_rl_repo",):
    if _p not in sys.path:
        sys.path.insert(0, _p)
import numpy as np
from contextlib import ExitStack

import concourse.bass as bass
import concourse.bacc as bacc
import concourse.tile as tile
from concourse import mybir

F32 = mybir.dt.float32
F32R = mybir.dt.float32r
BF16 = mybir.dt.bfloat16
AF = mybir.ActivationFunctionType
OP = mybir.AluOpType

C, DI, N, R, K = 96, 192, 16, 6, 4
HH = WW = 96
L_FULL = HH * WW     # 9216
EPS = 1e-5

TC = 768             # time chunk
SUB = 384            # psum sub-chunk

# engine-assignment knobs for the scan inner loop: (n*2+hf) pairs with
# idx < HC_DVE_K run the h*C multiply on DVE, the rest on Pool; pairs with
# idx < BT_DVE_K run the dtx*B multiply on DVE, the rest on Pool.
HC_DVE_K = 0
BT_DVE_K = 32


def _fix_act_tables(nc):
    """Replace the alternating exp/ln table loads with a single load of a
    set containing every activation function the program uses."""
    from concourse.hw_specs import get_activation_tables
    used = set()
    for b in nc.main_func.blocks:
        for i in b.instructions:
            if isinstance(i, mybir.InstActivation):
                used.add(i.func)
    tables = get_activation_tables(nc.m.arch)
    target = None
    for idx, (name, funcs) in enumerate(tables.items()):
        if used <= funcs:
            target = idx
            break
    if target is None:
        return  # no single covering set; leave the compiler's placement
    for b in nc.main_func.blocks:
        keep_done = False
        new_insts = []
        for i in b.instructions:
            if isinstance(i, mybir.InstLoadActFuncSet):
                si = i.sync_info
                clean = si is None or (len(si.on_wait) == 0
                                       and len(si.on_update) == 0)
                if not keep_done or not clean:
                    i.act_func_set_id = target
                    keep_done = True
                    new_insts.append(i)
                # else: drop redundant load
            else:
                new_insts.append(i)
        b.instructions[:] = new_insts


def build_program(L=L_FULL, Tc=TC, pow_dA=False):
    NCHUNK = L // Tc
    NSUB = Tc // SUB
    TC3 = Tc + 3
    nc = bacc.Bacc("TRN2", target_bir_lowering=False, debug=False)

    x_in = nc.dram_tensor("x", [C, L], F32, kind="ExternalInput")
    wIN = nc.dram_tensor("wIN", [C, K * DI], F32, kind="ExternalInput")
    wZ = nc.dram_tensor("wZ", [C, DI], F32, kind="ExternalInput")
    wXPb = nc.dram_tensor("wXPb", [C, 2 * 38], BF16, kind="ExternalInput")
    wDTb = nc.dram_tensor("wDTb", [R, DI], BF16, kind="ExternalInput")
    wA2 = nc.dram_tensor("wA2", [128, 24], F32, kind="ExternalInput")
    wOUTb = nc.dram_tensor("wOUTb", [C, 2 * C], BF16, kind="ExternalInput")
    vec2 = nc.dram_tensor("vec2", [C, 8], F32, kind="ExternalInput")
    vec1 = nc.dram_tensor("vec1", [C, 3], F32, kind="ExternalInput")
    gate_in = nc.dram_tensor("gate", [1, 1], F32, kind="ExternalInput")
    red_in = nc.dram_tensor("red", [128, 3 * C], BF16, kind="ExternalInput")

    # ping-pong staging for the per-chunk B/C and dt/dtx rows
    # (replication/broadcast-DMA sources)
    bc_d = [nc.dram_tensor(f"bcrows{p}", [2 * N, Tc], BF16, kind="Internal")
            for p in range(2)]
    dt_d = [nc.dram_tensor(f"dtrows{p}", [2 * C, Tc], BF16, kind="Internal")
            for p in range(2)]
    dx_d = [nc.dram_tensor(f"dxrows{p}", [2 * C, Tc], BF16, kind="Internal")
            for p in range(2)]

    p_out = nc.dram_tensor("p", [C, L], F32, kind="ExternalOutput")

    with ExitStack() as ctx:
        tc = ctx.enter_context(tile.TileContext(nc))
        wp = ctx.enter_context(tc.tile_pool(name="wts", bufs=1))
        px = ctx.enter_context(tc.tile_pool(name="px", bufs=3))
        ph = ctx.enter_context(tc.tile_pool(name="ph", bufs=3))
        pt0 = ctx.enter_context(tc.tile_pool(name="pt0", bufs=2))
        pt1 = ctx.enter_context(tc.tile_pool(name="pt1", bufs=2))
        psp = ctx.enter_context(tc.tile_pool(name="psp", bufs=2))
        prow = ctx.enter_context(tc.tile_pool(name="prow", bufs=2))
        pxc = ctx.enter_context(tc.tile_pool(name="pxc", bufs=2))
        pg = ctx.enter_context(tc.tile_pool(name="pg", bufs=2))
        pdbl = ctx.enter_context(tc.tile_pool(name="pdbl", bufs=2))
        pdt = ctx.enter_context(tc.tile_pool(name="pdt", bufs=2))
        pdtx = ctx.enter_context(tc.tile_pool(name="pdtx", bufs=2))
        pbb = ctx.enter_context(tc.tile_pool(name="pbb", bufs=2))
        psc = ctx.enter_context(tc.tile_pool(name="psc", bufs=2))
        phh = ctx.enter_context(tc.tile_pool(name="phh", bufs=2))
        phc = ctx.enter_context(tc.tile_pool(name="phc", bufs=2))
        pst = ctx.enter_context(tc.tile_pool(name="pst", bufs=1))
        ptail = ctx.enter_context(tc.tile_pool(name="ptail", bufs=2))
        ppr = ctx.enter_context(tc.tile_pool(name="ppr", bufs=1))

        qh = ctx.enter_context(tc.tile_pool(name="qh", bufs=2, space="PSUM"))
        qdbl = ctx.enter_context(tc.tile_pool(name="qdbl", bufs=2,
                                              space="PSUM"))
        qy = ctx.enter_context(tc.tile_pool(name="qy", bufs=1, space="PSUM"))

        w_in = wp.tile([C, K * DI], F32); nc.sync.dma_start(w_in[:], wIN[:])
        w_z = wp.tile([C, DI], F32); nc.sync.dma_start(w_z[:], wZ[:])
        w_xpb = wp.tile([C, 2 * 38], BF16); nc.sync.dma_start(w_xpb[:],
                                                             wXPb[:])
        w_dtb = wp.tile([R, DI], BF16); nc.sync.dma_start(w_dtb[:], wDTb[:])
        w_a2 = wp.tile([128, 24], F32); nc.sync.dma_start(w_a2[:], wA2[:])
        w_outb = wp.tile([C, 2 * C], BF16); nc.sync.dma_start(w_outb[:],
                                                             wOUTb[:])
        v2 = wp.tile([C, 8], F32); nc.sync.dma_start(v2[:], vec2[:])
        v1 = wp.tile([C, 3], F32); nc.sync.dma_start(v1[:], vec1[:])
        gt = wp.tile([1, 1], F32); nc.sync.dma_start(gt[:], gate_in[:])

        # fp32r-rounded copies of the stationary matmul operands
        w_inr = wp.tile([C, K * DI], F32R); nc.scalar.copy(w_inr[:], w_in[:])
        w_zr = wp.tile([C, DI], F32R); nc.scalar.copy(w_zr[:], w_z[:])
        red_b = wp.tile([128, 3 * C], BF16)
        nc.sync.dma_start(red_b[:], red_in[:])

        ones_col = wp.tile([C, 1], F32); nc.vector.memset(ones_col[:], 1.0)
        ones_col_r = wp.tile([C, 1], F32R); nc.scalar.copy(ones_col_r[:],
                                                          ones_col[:])
        ones_row = wp.tile([1, C], F32); nc.vector.memset(ones_row[:], 1.0)
        ones_row_r = wp.tile([1, C], F32R); nc.scalar.copy(ones_row_r[:],
                                                          ones_row[:])
        epsc = wp.tile([1, 1], F32); nc.vector.memset(epsc[:], EPS)
        gate_c = wp.tile([C, 1], F32)
        qg = qh.tile([C, 1], F32, tag="a")
        nc.tensor.matmul(qg[:], ones_row[:], gt[:])
        nc.scalar.copy(gate_c[:], qg[:])

        # probes: absorb cross-engine waits (TensorScalarPtr ops: 1 wait slot)
        prv = ppr.tile([1, 8], F32)
        pra = ppr.tile([1, 8], F32)
        nc.vector.tensor_copy(prv[:, 0:1], v1[:1, 0:1])
        nc.vector.tensor_copy(prv[:, 1:2], v2[:1, 0:1])
        nc.vector.tensor_copy(prv[:, 2:3], gate_c[:1, 0:1])
        nc.scalar.copy(pra[:, 0:1], w_a2[:1, 0:1])
        nc.scalar.copy(pra[:, 1:2], v1[:1, 0:1])
        nc.scalar.copy(pra[:, 2:3], v2[:1, 0:1])

        st = pst.tile([128, 24], BF16)
        nc.vector.memset(st[:], 0.0)
        zero3 = wp.tile([C, 3], F32); nc.vector.memset(zero3[:], 0.0)

        def make_prologue(ci, chunks):
            """Emit-later closures for chunk ci's pre-scan pipeline. Each
            stage is emitted interleaved with the previous chunk's scan
            iterations so the in-order engine queues overlap them."""
            S = {"t0": ci * Tc}
            t0 = S["t0"]

            def s_dma():
                S["xt"] = px.tile([C, Tc], F32, tag="xt", name="xt")
                nc.sync.dma_start(S["xt"][:], x_in[:, t0:t0 + Tc])

            def s_sq():
                S["sq"] = pt0.tile([C, Tc], F32R, tag="sql", name="sq")
                nc.scalar.activation(S["sq"][:], S["xt"][:], AF.Square)

            def s_stats():
                S["m_"] = prow.tile([1, Tc], F32R, tag="m", bufs=2, name="m_")
                S["var_"] = prow.tile([1, Tc], F32, tag="var", bufs=1,
                                      name="var_")
                mm_ = prow.tile([1, Tc], F32, tag="mm", bufs=1, name="mm_")
                for si in range(NSUB):
                    o = si * SUB
                    s1 = qh.tile([1, SUB], F32, tag="a", name="s1")
                    nc.tensor.matmul(s1[:], ones_col[:], S["xt"][:, o:o + SUB])
                    nc.vector.tensor_scalar_mul(
                        S["m_"][:, o:o + SUB], s1[:], 1.0 / C)
                    s2 = qh.tile([1, SUB], F32, tag="a", name="s2")
                    nc.tensor.matmul(s2[:], ones_col_r[:],
                                     S["sq"][:, o:o + SUB])
                    nc.vector.tensor_tensor(
                        mm_[:, o:o + SUB], S["m_"][:, o:o + SUB].bitcast(F32),
                        S["m_"][:, o:o + SUB].bitcast(F32), op=OP.mult)
                    nc.vector.scalar_tensor_tensor(
                        S["var_"][:, o:o + SUB], s2[:], 1.0 / C,
                        mm_[:, o:o + SUB], op0=OP.mult, op1=OP.subtract)

            def s_rstd():
                lnv = prow.tile([1, Tc], F32, tag="lnv", bufs=1, name="lnv")
                S["rstd"] = prow.tile([1, Tc], F32R, tag="rstd", bufs=2,
                                      name="rstd")
                nc.scalar.activation(lnv[:], S["var_"][:], AF.Ln,
                                     bias=epsc[:, 0:1])
                nc.scalar.activation(S["rstd"][:], lnv[:], AF.Exp, scale=-0.5)

            def s_xn():
                S["xn"] = pt1.tile([C, Tc], F32, tag="xn", name="xn")
                for si in range(NSUB):
                    o = si * SUB
                    mb = qh.tile([C, SUB], F32, tag="a", name="mb")
                    nc.tensor.matmul(mb[:], ones_row_r[0:1, :],
                                     S["m_"][:, o:o + SUB])
                    nc.vector.tensor_tensor(S["xn"][:, o:o + SUB],
                                            S["xt"][:, o:o + SUB],
                                            mb[:], op=OP.subtract)
                    rb = qh.tile([C, SUB], F32, tag="a", name="rb")
                    nc.tensor.matmul(rb[:], ones_row_r[0:1, :],
                                     S["rstd"][:, o:o + SUB])
                    nc.vector.tensor_tensor(S["xn"][:, o:o + SUB],
                                            S["xn"][:, o:o + SUB],
                                            rb[:], op=OP.mult)

            def s_ln():
                S["ln_t"] = pt1.tile([C, Tc], F32, tag="ln", name="ln_t")
                nc.scalar.activation(S["ln_t"][:], S["xn"][:], AF.Identity,
                                     bias=v1[:, 1:2], scale=v1[:, 0:1])
                S["lsq"] = pt0.tile([C, Tc], F32R, tag="sql", name="lsq")
                nc.scalar.activation(S["lsq"][:], S["ln_t"][:], AF.Square)

            def s_rr():
                lnr = prow.tile([1, Tc], F32, tag="lnr", bufs=1, name="lnr")
                S["rr"] = prow.tile([1, Tc], F32R, tag="rr", bufs=2, name="rr")
                for si in range(NSUB):
                    o = si * SUB
                    s3 = qh.tile([1, SUB], F32, tag="a", name="s3")
                    nc.tensor.matmul(s3[:], ones_col_r[:],
                                     S["lsq"][:, o:o + SUB])
                    nc.scalar.activation(lnr[:, o:o + SUB], s3[:],
                                         AF.Ln, scale=1.0 / C,
                                         bias=epsc[:, 0:1])
                    nc.scalar.activation(S["rr"][:, o:o + SUB],
                                         lnr[:, o:o + SUB], AF.Exp, scale=-0.5)

            def s_h():
                h_t = ph.tile([C, TC3], F32R, tag="h", name="h_t")
                S["h_t"] = h_t
                if ci == 0:
                    nc.scalar.copy(h_t[:, 0:3], zero3[:])
                else:
                    hp = chunks[ci - 1]["h_t"]
                    nc.vector.tensor_copy(h_t[:, 0:3],
                                          hp[:, Tc:Tc + 3].bitcast(F32))
                nc.vector.tensor_copy(prv[:, 3:4], S["ln_t"][:1, 0:1])
                for si in range(NSUB):
                    o = si * SUB
                    rrb = qh.tile([C, SUB], F32, tag="a", name="rrb")
                    nc.tensor.matmul(rrb[:], ones_row_r[0:1, :],
                                     S["rr"][:, o:o + SUB])
                    nc.vector.scalar_tensor_tensor(
                        h_t[:, 3 + o:3 + o + SUB], S["ln_t"][:, o:o + SUB],
                        v1[:, 2:3], rrb[:], op0=OP.mult, op1=OP.mult)

            def make_conv(hf, si):
                def s_conv():
                    if "xc_h" not in S:
                        S["xc_h"] = [None, None]
                        S["g_h"] = [None, None]
                    if S["xc_h"][hf] is None:
                        S["xc_h"][hf] = pxc.tile([C, Tc], BF16, tag=f"xc{hf}",
                                                 name=f"xc{hf}")
                        S["g_h"][hf] = pg.tile([C, Tc], BF16, tag=f"g{hf}",
                                               name=f"g{hf}")
                    xc = S["xc_h"][hf]
                    g = S["g_h"][hf]
                    h_t = S["h_t"]
                    if True:
                        o = si * SUB
                        ps = qh.tile([C, SUB], F32, tag="a", name="psc1")
                        for k in range(K):
                            nc.tensor.matmul(
                                ps[:],
                                w_inr[:, k * DI + hf * C:k * DI + hf * C + C],
                                h_t[:, o + k:o + k + SUB],
                                start=(k == 0), stop=(k == K - 1))
                        # silu(p+cb) = (p+cb) / (1+exp(-(p+cb)))
                        e1 = psp.tile([C, SUB], F32, tag="sg1", name="e1")
                        nc.scalar.activation(e1[:], ps[:], AF.Exp, scale=-1.0,
                                             bias=v2[:, 6 + hf:7 + hf])
                        f1 = psp.tile([C, SUB], F32, tag="sg3", name="f1")
                        nc.scalar.activation(f1[:], e1[:], AF.Identity,
                                             bias=1.0)
                        r1 = psp.tile([C, SUB], F32, tag="sg2", name="r1")
                        nc.vector.reciprocal_approx_fast(r1[:], f1[:])
                        nc.vector.scalar_tensor_tensor(
                            xc[:, o:o + SUB], ps[:], v2[:, hf:hf + 1], r1[:],
                            op0=OP.add, op1=OP.mult)
                        ps2 = qh.tile([C, SUB], F32, tag="a", name="psc2")
                        nc.tensor.matmul(ps2[:], w_zr[:, hf * C:hf * C + C],
                                         h_t[:, o + 3:o + 3 + SUB])
                        e2 = psp.tile([C, SUB], F32, tag="sg1", name="e2")
                        nc.scalar.activation(e2[:], ps2[:], AF.Exp, scale=-1.0)
                        f2 = psp.tile([C, SUB], F32, tag="sg3", name="f2")
                        nc.scalar.activation(f2[:], e2[:], AF.Identity,
                                             bias=1.0)
                        r2 = psp.tile([C, SUB], F32, tag="sg2", name="r2")
                        nc.vector.reciprocal_approx_fast(r2[:], f2[:])
                        nc.vector.tensor_tensor(g[:, o:o + SUB], ps2[:], r2[:],
                                                op=OP.mult)
                return s_conv

            def make_dbl(si):
                # B/C/dt projections in one PSUM tile; bf16 copy. Matmul
                # outputs must start at partition 0 or 32 and match the
                # lhsT base, so: rows [0:32) = dt-proj + a redundant B/C
                # prefix (keeps every copied row defined), rows [32:64) =
                # B/C. dt-proj = dbc[0:R], B = dbc[32:48], C = dbc[48:64].
                def s_dbl():
                    if "dbc" not in S:
                        S["dbc"] = pdbl.tile([64, Tc], BF16, tag="dbc",
                                             name="dbc")
                    o = si * SUB
                    ps = qdbl.tile([64, SUB], F32, tag="d", name="psdbl")
                    for lo, src_lo in ((0, 0), (32, R)):
                        for hf in range(2):
                            nc.tensor.matmul(
                                ps[lo:lo + 32, :],
                                w_xpb[:, hf * 38 + src_lo:
                                      hf * 38 + src_lo + 32],
                                S["xc_h"][hf][:, o:o + SUB],
                                start=(hf == 0), stop=(hf == 1))
                    nc.scalar.copy(S["dbc"][:, o:o + SUB], ps[:])
                    if si == NSUB - 1:
                        # stage B/C rows to DRAM for the broadcast reads
                        nc.sync.dma_start(bc_d[ci % 2][:, :],
                                          S["dbc"][32:64, :])
                return s_dbl

            def make_dt(hf):
                def s_dt():
                    if "dt_h" not in S:
                        S["dt_h"] = [None, None]
                    S["dt_h"][hf] = pdt.tile([C, Tc], BF16, tag=f"dt{hf}",
                                             name=f"dt{hf}")
                    for si in range(NSUB):
                        o = si * SUB
                        ps = qh.tile([C, SUB], F32, tag="a", name="psdt")
                        nc.tensor.matmul(ps[:], w_dtb[:, hf * C:hf * C + C],
                                         S["dbc"][0:R, o:o + SUB])
                        # softplus: dt projections sit near dtb ~ -4, so the
                        # direct ln(1+exp(v)) form cannot overflow
                        ex = psp.tile([C, SUB], F32, tag="spe", name="ex")
                        nc.scalar.activation(ex[:], ps[:], AF.Exp,
                                             bias=v2[:, 2 + hf:3 + hf])
                        nc.scalar.activation(S["dt_h"][hf][:, o:o + SUB],
                                             ex[:], AF.Ln, bias=1.0)
                    nc.sync.dma_start(dt_d[ci % 2][hf * C:(hf + 1) * C, :],
                                      S["dt_h"][hf][:])
                return s_dt

            def make_dtx(hf):
                def s_dtx():
                    dx = pdtx.tile([C, Tc], BF16, tag=f"dtx{hf}",
                                   name=f"dtx{hf}")
                    nc.vector.tensor_tensor(dx[:], S["dt_h"][hf][:],
                                            S["xc_h"][hf][:], op=OP.mult)
                    nc.sync.dma_start(dx_d[ci % 2][hf * C:(hf + 1) * C, :],
                                      dx[:])
                return s_dtx

            def _rep_from(dram, r0):
                # (128,Tc) <- rows [r0, r0+32) of `dram`, each row at the 4
                # partitions a*32+j (a = n-subindex, j = d-subindex)
                sl = dram[r0:r0 + 32, :]
                return bass.AP(tensor=sl.tensor, offset=sl.offset,
                               ap=[[0, 4]] + [list(a) for a in sl.ap])

            def make_rep(hf):
                def s_rep():
                    if "dtp" not in S:
                        S["dtp"] = [[None] * 3, [None] * 3]
                        S["dxp"] = [[None] * 3, [None] * 3]
                    for g_d in range(3):
                        tp = pdt.tile([128, Tc], BF16, tag=f"dtp{hf}{g_d}",
                                      name=f"dtp{hf}{g_d}")
                        nc.sync.dma_start(
                            tp[:], _rep_from(dt_d[ci % 2], hf * C + 32 * g_d))
                        S["dtp"][hf][g_d] = tp
                        xp = pdtx.tile([128, Tc], BF16, tag=f"dxp{hf}{g_d}",
                                       name=f"dxp{hf}{g_d}")
                        nc.sync.dma_start(
                            xp[:], _rep_from(dx_d[ci % 2], hf * C + 32 * g_d))
                        S["dxp"][hf][g_d] = xp
                return s_rep

            early = [s_dma, s_sq, s_stats, s_rstd, s_xn, s_ln, s_rr, s_h]
            late = [make_conv(0, 0), make_conv(0, 1), make_conv(1, 0),
                    make_conv(1, 1), make_dbl(0), make_dbl(1),
                    make_dt(0), make_dt(1), make_dtx(0), make_dtx(1),
                    make_rep(0), make_rep(1)]
            return S, early, late

        def emit_scan_and_tail(S, ci, nxt_stages, nxt_S=None):
            t0 = S["t0"]
            xt, xc_h, g_h = S["xt"], S["xc_h"], S["g_h"]
            dtp, dxp = S["dtp"], S["dxp"]
            yps = [[qy.tile([C, SUB], F32, tag=f"y{hf}_{si}",
                            name=f"y{hf}_{si}")
                    for si in range(NSUB)] for hf in range(2)]

            def emit_bc(g_n):
                # (128,Tc) b/c tiles: rows 4*g_n+a at partitions a*32+j
                bb = pbb.tile([128, Tc], BF16, tag="bb", name="bb", bufs=4)
                cb = pbb.tile([128, Tc], BF16, tag="cb", name="cb", bufs=4)
                src_ = bc_d[ci % 2]
                for dst, r0 in ((bb, 4 * g_n), (cb, N + 4 * g_n)):
                    sl = src_[r0:r0 + 4, :]
                    ap = [list(sl.ap[0]), [0, 32], list(sl.ap[1])]
                    nc.sync.dma_start(dst[:], bass.AP(
                        tensor=sl.tensor, offset=sl.offset, ap=ap))
                return bb, cb

            iters = [(g_n, hf, g_d) for g_n in range(4) for hf in range(2)
                     for g_d in range(3)]

            def emit_dA(it):
                g_n, hf, g_d = iters[it]
                dA = psc.tile([128, Tc], F32, tag="dA", name="dA", bufs=4)
                nc.scalar.activation(dA[:], dtp[hf][g_d][:], AF.Exp,
                                     scale=w_a2[:, it:it + 1])
                return dA

            S["emit_bc"] = emit_bc
            S["emit_dA"] = emit_dA
            bcq = [S.pop("bc0", None) or emit_bc(0), emit_bc(1)]
            pre = S.pop("dA01", None)
            dAq = pre if pre else [emit_dA(0), emit_dA(1)]
            dAq.append(emit_dA(2))
            for it, (g_n, hf, g_d) in enumerate(iters):
                if g_d == 0 and hf == 0:
                    bb, cb = bcq.pop(0)
                    if g_n + 2 < 4:
                        bcq.append(emit_bc(g_n + 2))
                dA = dAq.pop(0)
                if it + 3 < 24:
                    dAq.append(emit_dA(it + 3))
                bt = psc.tile([128, Tc], BF16, tag="bt", name="bt", bufs=4)
                nc.vector.tensor_tensor(bt[:], dxp[hf][g_d][:], bb[:],
                                        op=OP.mult)
                ht = phh.tile([128, Tc], BF16, tag="ht", name="ht", bufs=4)
                nc.vector.tensor_tensor_scan(
                    ht[:], dA[:], bt[:], st[:, it:it + 1],
                    op0=OP.mult, op1=OP.add)
                nc.vector.tensor_copy(st[:, it:it + 1], ht[:, Tc - 1:Tc])
                hc = phc.tile([128, Tc], BF16, tag="hc", name="hc", bufs=4)
                nc.gpsimd.tensor_tensor(hc[:], ht[:], cb[:], op=OP.mult)
                first = (g_n == 0 and g_d == 0)
                last = (g_n == 3 and g_d == 2)
                for si in range(NSUB):
                    o = si * SUB
                    nc.tensor.matmul(yps[hf][si][:],
                                     red_b[:, g_d * C:(g_d + 1) * C],
                                     hc[:, o:o + SUB],
                                     start=first, stop=last,
                                     skip_group_check=True)
                if nxt_stages:
                    nxt_stages.pop(0)()

            while nxt_stages:
                nxt_stages.pop(0)()
            yg_h = []
            for hf in range(2):
                ya = ptail.tile([C, Tc], BF16, tag=f"ya{hf}", name=f"ya{hf}",
                                bufs=1)
                for si in range(NSUB):
                    o = si * SUB
                    nc.vector.scalar_tensor_tensor(
                        ya[:, o:o + SUB], xc_h[hf][:, o:o + SUB],
                        v2[:, 4 + hf:5 + hf], yps[hf][si][:],
                        op0=OP.mult, op1=OP.add)
                yg = ptail.tile([C, Tc], BF16, tag=f"yg{hf}", name=f"yg{hf}",
                                bufs=1)
                nc.gpsimd.tensor_tensor(yg[:], ya[:], g_h[hf][:], op=OP.mult)
                yg_h.append(yg)
            ot = ptail.tile([C, Tc], F32, tag="ot", name="ot")
            for si in range(NSUB):
                o = si * SUB
                pso = qh.tile([C, SUB], F32, tag="a", name="pso")
                for hf in range(2):
                    nc.tensor.matmul(pso[:], w_outb[:, hf * C:hf * C + C],
                                     yg_h[hf][:, o:o + SUB],
                                     start=(hf == 0), stop=(hf == 1))
                nc.vector.scalar_tensor_tensor(
                    ot[:, o:o + SUB], xt[:, o:o + SUB], gate_c[:, 0:1],
                    pso[:], op0=OP.mult, op1=OP.add)
            nc.sync.dma_start(p_out[:, t0:t0 + Tc], ot[:])

        chunks = {}
        S0, early0, late0 = make_prologue(0, chunks)
        chunks[0] = S0
        for f in early0 + late0:
            f()
        pend_late = {}
        if NCHUNK > 1:
            S1, early1, late1 = make_prologue(1, chunks)
            chunks[1] = S1
            for f in early1:
                f()
            pend_late[1] = late1
        for ci in range(NCHUNK):
            stages = list(pend_late.pop(ci + 1, []))
            if ci + 2 < NCHUNK:
                S2, early2, late2 = make_prologue(ci + 2, chunks)
                chunks[ci + 2] = S2
                stages += early2
                pend_late[ci + 2] = late2
            emit_scan_and_tail(chunks[ci], ci, stages,
                               chunks.get(ci + 1))
            chunks.pop(ci - 1, None)

    nc.compile()
    _fix_act_tables(nc)
    return nc


# ---------------------------------------------------------------- host side

def shuffle_channels(x):
    c = x.shape[0]
    return x.reshape(2, c // 2, -1).transpose(1, 0, 2).reshape(c, -1)


def _bf16():
    try:
        import ml_dtypes
        return ml_dtypes.bfloat16
    except Exception:
        import jax.numpy as _jnp
        return _jnp.bfloat16


def pack_core_inputs(i, dr, b, x1, x2, inw, convw, convb, xpw, dtw, dtb,
                     Alog, Dp, outw, rmsw, lnw, lnb):
    xs = x1 if i == 0 else x2
    x = shuffle_channels(np.asarray(xs[b], np.float32))
    if dr == 1:
        x = x[:, ::-1]
    x = np.ascontiguousarray(x)

    inw_i = np.asarray(inw[i], np.float32)
    cw = np.asarray(convw[i, dr], np.float32)
    cb = np.asarray(convb[i, dr], np.float32)
    xp = np.asarray(xpw[i, dr], np.float32)
    dw = np.asarray(dtw[i, dr], np.float32)
    db = np.asarray(dtb[i, dr], np.float32)
    Av = -np.exp(np.asarray(Alog[i, dr], np.float32))
    Dv = np.asarray(Dp[i, dr], np.float32)
    ow = np.asarray(outw[i], np.float32)

    wIN = np.empty((C, K * DI), np.float32)
    inw_x = inw_i[:DI]
    for k in range(K):
        wIN[:, k * DI:(k + 1) * DI] = (cw[:, k][:, None] * inw_x).T
    wZ = np.ascontiguousarray(inw_i[DI:].T)
    bf16 = _bf16()
    wXP = np.empty((C, 2 * 38), np.float32)
    for hf in range(2):
        wXP[:, hf * 38:(hf + 1) * 38] = xp[:, hf * C:(hf + 1) * C].T
    wXPb = wXP.astype(bf16)
    wDTb = np.ascontiguousarray(dw.T).astype(bf16)
    # packed-layout A scales: column it=(g_n*6+hf*3+g_d) holds, at
    # partition p=a*32+j, the A value for n=4*g_n+a, d=32*g_d+j
    wA2 = np.empty((128, 24), np.float32)
    for g_n in range(4):
        for hf in range(2):
            for g_d in range(3):
                it = g_n * 6 + hf * 3 + g_d
                for a in range(4):
                    for j in range(32):
                        wA2[a * 32 + j, it] = Av[hf * C + 32 * g_d + j,
                                                 4 * g_n + a]
    wOUT = np.empty((C, 2 * C), np.float32)
    for hf in range(2):
        wOUT[:, hf * C:(hf + 1) * C] = 0.5 * ow[:, hf * C:(hf + 1) * C].T
    wOUTb = wOUT.astype(bf16)
    vec2 = np.ascontiguousarray(
        np.stack([cb[:C], cb[C:], db[:C], db[C:], Dv[:C], Dv[C:],
                  -cb[:C], -cb[C:]], axis=1), dtype=np.float32)
    vec1 = np.ascontiguousarray(
        np.stack([np.asarray(lnw[i], np.float32),
                  np.asarray(lnb[i], np.float32),
                  np.asarray(rmsw[i], np.float32)], axis=1), dtype=np.float32)
    gate = np.array([[1.0 if dr == 0 else 0.0]], np.float32)
    red = np.zeros((128, 3 * C), np.float32)
    for g_d in range(3):
        for a in range(4):
            for j in range(32):
                red[a * 32 + j, g_d * C + 32 * g_d + j] = 1.0
    red = red.astype(bf16)
    return {
        "x": x, "wIN": wIN, "wZ": wZ, "wXPb": wXPb, "wDTb": wDTb,
        "wA2": wA2, "wOUTb": wOUTb, "vec2": vec2, "vec1": vec1,
        "gate": gate, "red": red,
    }


def make_in_maps(inputs):
    args = dict(
        x1=np.asarray(inputs["x1"], np.float32),
        x2=np.asarray(inputs["x2"], np.float32),
        inw=np.asarray(inputs["inw"], np.float32),
        convw=np.asarray(inputs["convw"], np.float32),
        convb=np.asarray(inputs["convb"], np.float32),
        xpw=np.asarray(inputs["xpw"], np.float32),
        dtw=np.asarray(inputs["dtw"], np.float32),
        dtb=np.asarray(inputs["dtb"], np.float32),
        Alog=np.asarray(inputs["Alog"], np.float32),
        Dp=np.asarray(inputs["Dp"], np.float32),
        outw=np.asarray(inputs["outw"], np.float32),
        rmsw=np.asarray(inputs["rmsw"], np.float32),
        lnw=np.asarray(inputs["lnw"], np.float32),
        lnb=np.asarray(inputs["lnb"], np.float32),
    )
    in_maps, core_meta = [], []
    for i in range(2):
        for dr in range(2):
            for b in range(2):
                in_maps.append(pack_core_inputs(i, dr, b, **args))
                core_meta.append((i, dr, b))
    return in_maps, core_meta


def assemble_outputs(results, core_meta):
    B = 2
    outs = []
    for i in range(2):
        acc = np.zeros((B, C, L_FULL), np.float32)
        for (ii, dr, b), res in zip(core_meta, results):
            if ii != i:
                continue
            p = res["p"]
            if dr == 1:
                p = p[:, ::-1]
            acc[b] += p
        outs.append(acc.reshape(B, C, HH, WW))
    return tuple(outs)


# ------------------------------------------------------------- PJRT executor

class _BassExec:
    def __init__(self, nc, n_cores):
        import jax
        from jax.sharding import Mesh, PartitionSpec
        from jax.experimental.shard_map import shard_map
        from concourse.bass2jax import (_bass_exec_p, install_neuronx_cc_hook,
                                        partition_id_tensor)
        install_neuronx_cc_hook()
        self.jax = jax
        self.n_cores = n_cores
        partition_name = (nc.partition_id_tensor.name
                          if nc.partition_id_tensor else None)
        in_names, out_names, out_avals, zero_outs = [], [], [], []
        for alloc in nc.m.functions[0].allocations:
            if not isinstance(alloc, mybir.MemoryLocationSet):
                continue
            name = alloc.memorylocations[0].name
            if alloc.kind == "ExternalInput":
                if name != partition_name:
                    in_names.append(name)
            elif alloc.kind == "ExternalOutput":
                shape = tuple(alloc.tensor_shape)
                dtype = mybir.dt.np(alloc.dtype)
                out_names.append(name)
                out_avals.append(jax.core.ShapedArray(shape, dtype))
                zero_outs.append(np.zeros(shape, dtype))
        self.in_names, self.out_names = in_names, out_names
        self.out_avals, self.zero_outs = out_avals, zero_outs
        n_params, n_outs = len(in_names), len(out_avals)
        bind_names = in_names + out_names + ([partition_name] if partition_name
                                             else [])

        def _body(*args):
            operands = list(args)
            if partition_name is not None:
                operands.append(partition_id_tensor())
            outs = _bass_exec_p.bind(
                *operands,
                out_avals=tuple(out_avals),
                in_names=tuple(bind_names),
                out_names=tuple(out_names),
                lowering_input_output_aliases=(),
                sim_require_finite=True,
                sim_require_nnan=True,
                nc=nc,
            )
            return tuple(outs)

        devices = jax.devices()[:n_cores]
        self.mesh = Mesh(np.asarray(devices), ("core",))
        in_specs = (PartitionSpec("core"),) * (n_params + n_outs)
        out_specs = (PartitionSpec("core"),) * n_outs
        self.fn = jax.jit(
            shard_map(_body, mesh=self.mesh, in_specs=in_specs,
                      out_specs=out_specs, check_rep=False),
            keep_unused=True)

    def prep(self, in_maps):
        from jax.sharding import NamedSharding, PartitionSpec
        concat_in = [
            np.concatenate([np.asarray(in_maps[c][n])
                            for c in range(self.n_cores)], axis=0)
            for n in self.in_names
        ]
        concat_zero = [
            np.zeros((self.n_cores * z.shape[0], *z.shape[1:]), z.dtype)
            for z in self.zero_outs
        ]
        sh = NamedSharding(self.mesh, PartitionSpec("core"))
        return [self.jax.device_put(a, sh) for a in concat_in + concat_zero]

    def run(self, args):
        outs = self.fn(*args)
        self.jax.block_until_ready(outs)
        return outs

    def results(self, outs):
        res = []
        for c in range(self.n_cores):
            m = {}
            for i, name in enumerate(self.out_names):
                a = np.asarray(outs[i])
                a = a.reshape(self.n_cores, *self.out_avals[i].shape)[c]
                m[name] = a
            res.append(m)
        return res


_CACHE = {}


def _get_exec(pow_dA=False):
    key = f"ex{int(pow_dA)}"
    if key not in _CACHE:
        nc = build_program(pow_dA=pow_dA)
        _CACHE[key] = _BassExec(nc, 8)
    return _CACHE[key]


def kernel(**inputs):
    H = int(inputs.get("H", HH))
    W = int(inputs.get("W", WW))
    assert H == HH and W == WW, (H, W)
    in_maps, core_meta = make_in_maps(inputs)
    ex = _get_exec(pow_dA=False)
    args = ex.prep(in_maps)
    outs = ex.run(args)
    res = ex.results(outs)
    return assemble_outputs(res, core_meta)
